# revision 1
# baseline (speedup 1.0000x reference)
"""Trainium2 Bass kernel for nn_MLDecoder (moe_routing).

Data-parallel over batch across 8 NeuronCores (32 batch rows/core, head params
replicated). Activations stay feature-major ("transposed"): C^T = W^T A^T via
matmul(out=C^T, lhsT=W(natural), rhs=A^T). Rows r = b*100+g (b-major). The
batch-independent query path (tgt0, q) is constant-folded on the host. All
matmuls bf16 with fp32 PSUM; LN stats via ones-matmuls; softmax without
max-subtraction (scores are O(1) for this head).
"""
import sys
sys.path.insert(0, "/opt/trn_rl_repo")

import numpy as np
import ml_dtypes

import concourse.bass as bass
from concourse import bacc
import concourse.tile as tile
import concourse.mybir as mybir
from concourse.bass import AP
from concourse.bass_utils import run_bass_kernel_spmd
from concourse.masks import make_identity

F32 = mybir.dt.float32
BF16 = mybir.dt.bfloat16
BF = ml_dtypes.bfloat16
AF = mybir.ActivationFunctionType
ALU = mybir.AluOpType
AX = mybir.AxisListType

B, S, C0 = 256, 49, 2048
D, F = 768, 2048
G, DF = 100, 96
H, HD = 8, 96
EPS = 1e-5
NCORES = 8
BL = B // NCORES          # 32 batch rows per core
R = BL * G                # 3200 rows (b,g) per core
RC = 400                  # row chunk = 4 b
NCHUNK = R // RC
XCH = 4                   # x col chunks (8 b each)
XCOLS = (BL // XCH) * S   # 392
PADS = 64                 # padded spatial stride
MCOLS = BL * PADS         # 2048 padded mem cols


def _bf(a):
    return np.ascontiguousarray(a.astype(BF))


def _ap(base, free_dims):
    """Replace the free dims of a (sliced) AP, keeping its partition dim."""
    return AP(tensor=base.tensor, offset=base.offset,
              ap=[base.ap[0]] + [list(fd) for fd in free_dims])


def build_program(skip_dupb=False, ln_triv=False, ffn_triv=False):
    nc = bacc.Bacc("TRN2", target_bir_lowering=False, debug=False,
                   num_devices=NCORES)
    d = {}

    def din(name, shape, dt):
        d[name] = nc.dram_tensor(name, list(shape), dt, kind="ExternalInput").ap()

    din("xT", (XCH, 128, 16 * XCOLS), BF16)
    din("wemb", (128, 16 * 768), BF16)
    din("be", (128, 6), F32)
    din("wk", (128, 6 * 768), BF16)
    din("wv", (128, 6 * 768), BF16)
    din("wao", (96, 8 * 768), BF16)
    din("bao", (128, 6), F32)
    din("w1", (128, 6 * 2048), BF16)
    din("b1", (128, 16), F32)
    din("w2", (128, 16 * 768), BF16)
    din("b2", (128, 6), F32)
    din("qT", (96, 8 * 100), BF16)
    din("qbk", (100, 8), F32)
    din("tgt0", (128, 6 * 100), BF16)
    din("ln2g", (128, 6), F32)
    din("ln2b", (128, 6), F32)
    din("ln3g", (128, 6), F32)
    din("ln3b", (128, 6), F32)
    din("dup", (100, 128, 6 * 96), BF16)
    din("dupb", (1, G * 96), BF16)
    out_d = nc.dram_tensor("logitsT", [96, G * BL], F32,
                           kind="ExternalOutput").ap()

    with tile.TileContext(nc) as tc:
        build_kernel(tc, d, out_d, skip_dupb, ln_triv, ffn_triv)
    nc.compile()
    return nc


def build_kernel(tc, d, out_d, skip_dupb=False, ln_triv=False, ffn_triv=False):
    nc = tc.nc

    def pool(name, bufs=1, space="SBUF"):
        return tc.tile_pool(name=name, bufs=bufs, space=space)

    with pool("resident") as res, pool("h3pool") as h3p, pool("oTpool") as oTp:
        ident = res.tile([128, 128], BF16)
        make_identity(nc, ident[:])
        ones_col = res.tile([128, 1], BF16)
        nc.vector.memset(ones_col[:], 1.0)
        ones_row = res.tile([1, 128], BF16)
        nc.vector.memset(ones_row[:], 1.0)
        ones32 = res.tile([1, BL], BF16)
        nc.vector.memset(ones32[:], 1.0)
        eps_t = res.tile([1, 1], F32)
        nc.vector.memset(eps_t[:], EPS)

        small = {}
        for name in ["be", "bao", "b1", "b2", "qT", "qbk", "tgt0",
                     "ln2g", "ln2b", "ln3g", "ln3b"]:
            t = res.tile(list(d[name].shape), d[name].dtype, tag=name)
            nc.gpsimd.dma_start(out=t, in_=d[name])
            small[name] = t

        h3T = h3p.tile([128, 6 * R], BF16)
        rstd_all = h3p.tile([1, R], BF16)
        oT = oTp.tile([96, 8 * R], BF16)

        with pool("memTpool") as memp:
            memT = memp.tile([128, 6 * MCOLS], BF16)

            # ---- P0: mem^T = relu(We^T x^T + be), written b-padded ----
            with pool("p0w") as p0w, pool("p0x", bufs=2) as p0x, \
                 pool("p0ps", bufs=3, space="PSUM") as p0ps:
                wemb = p0w.tile([128, 16 * 768], BF16)
                nc.sync.dma_start(out=wemb, in_=d["wemb"])
                for c in range(XCH):
                    xt = p0x.tile([128, 16 * XCOLS], BF16)
                    nc.sync.dma_start(out=xt, in_=d["xT"][c])
                    for m in range(6):
                        ps = p0ps.tile([128, XCOLS], F32)
                        for k in range(16):
                            nc.tensor.matmul(
                                ps[:],
                                wemb[:, k * 768 + m * 128:k * 768 + m * 128 + 128],
                                xt[:, k * XCOLS:(k + 1) * XCOLS],
                                start=(k == 0), stop=(k == 15))
                        dst = _ap(memT[:, m * MCOLS + c * 8 * PADS:],
                                  [[PADS, 8], [1, S]])
                        src = _ap(ps[:], [[S, 8], [1, S]])
                        nc.scalar.activation(out=dst, in_=src, func=AF.Relu,
                                             bias=small["be"][:, m:m + 1],
                                             scale=1.0)

            # ---- P1: K^T (head-major, b-padded) and V (rows padded) ----
            with pool("kvpool") as kvp:
                KT = kvp.tile([96, 8 * MCOLS], BF16)
                Vp = kvp.tile([128, 16 * 768], BF16)
                with pool("p1w") as p1w, \
                     pool("p1ps", bufs=3, space="PSUM") as p1ps:
                    wk = p1w.tile([128, 6 * 768], BF16)
                    nc.sync.dma_start(out=wk, in_=d["wk"])
                    wv = p1w.tile([128, 6 * 768], BF16)
                    nc.sync.dma_start(out=wv, in_=d["wv"])
                    for h in range(H):
                        for c in range(XCH):
                            ps = p1ps.tile([96, XCOLS], F32)
                            for k in range(6):
                                rhs = _ap(memT[:, k * MCOLS + c * 8 * PADS:],
                                          [[PADS, 8], [1, S]])
                                nc.tensor.matmul(
                                    ps[:],
                                    wk[:, k * 768 + h * 96:k * 768 + h * 96 + 96],
                                    rhs, start=(k == 0), stop=(k == 5))
                            dst = _ap(KT[:, h * MCOLS + c * 8 * PADS:],
                                      [[PADS, 8], [1, S]])
                            nc.vector.tensor_copy(
                                out=dst, in_=_ap(ps[:], [[S, 8], [1, S]]))
                    for t in range(16):
                        ps = p1ps.tile([128, 768], F32)
                        for sub in range(2):
                            n0, n1 = sub * 512, min(768, (sub + 1) * 512)
                            for k in range(6):
                                nc.tensor.matmul(
                                    ps[:, n0:n1],
                                    memT[:, k * MCOLS + t * 128:
                                         k * MCOLS + t * 128 + 128],
                                    wv[:, k * 768 + n0:k * 768 + n1],
                                    start=(k == 0), stop=(k == 5))
                        nc.vector.tensor_copy(out=Vp[:, t * 768:(t + 1) * 768],
                                              in_=ps[:])

                # ---- P2: attention ----
                with pool("p2a", bufs=2) as p2a, pool("p2s", bufs=3) as p2s, \
                     pool("p2ps", bufs=2, space="PSUM") as psc, \
                     pool("p2pt", bufs=2, space="PSUM") as pst, \
                     pool("p2po", bufs=2, space="PSUM") as pso:
                    for bg in range(4):
                        attnT = p2a.tile([128, 8 * 400], BF16)
                        for h in range(H):
                            ps = psc.tile([100, 8 * S], F32)
                            rhs = _ap(KT[:, h * MCOLS + bg * 8 * PADS:],
                                      [[PADS, 8], [1, S]])
                            nc.tensor.matmul(ps[:],
                                             small["qT"][:, h * 100:(h + 1) * 100],
                                             rhs, start=True, stop=True)
                            # exp into 64-padded slots (pads hold garbage,
                            # excluded by every later access pattern)
                            att = p2s.tile([100, 8 * PADS], BF16)
                            nc.scalar.activation(out=_ap(att[:], [[PADS, 8], [1, S]]),
                                                 in_=ps[:],
                                                 func=AF.Exp,
                                                 bias=small["qbk"][:, h:h + 1],
                                                 scale=1.0)
                            sums = p2s.tile([100, 8], F32)
                            nc.vector.reduce_sum(out=sums[:],
                                                 in_=_ap(att[:], [[PADS, 8], [1, S]]),
                                                 axis=AX.X)
                            inv = p2s.tile([100, 8], F32)
                            nc.vector.reciprocal(out=inv[:], in_=sums[:])
                            attn = p2s.tile([100, 8 * PADS], BF16)
                            nc.vector.tensor_tensor(
                                out=_ap(attn[:], [[PADS, 8], [1, S]]),
                                in0=_ap(att[:], [[PADS, 8], [1, S]]),
                                in1=_ap(inv[:], [[1, 8], [0, S]]),
                                op=ALU.mult)
                            for pr in range(4):
                                pt = pst.tile([128, 100], BF16)
                                nc.tensor.transpose(
                                    pt[:], attn[:, pr * 128:(pr + 1) * 128],
                                    ident[0:100, 0:100])
                                nc.vector.tensor_copy(
                                    out=attnT[:, h * 400 + pr * 100:
                                              h * 400 + pr * 100 + 100],
                                    in_=pt[:])
                        for lb in range(8):
                            b = bg * 8 + lb
                            po = pso.tile([96, 1024], F32)
                            for h in range(H):
                                vsl = Vp[(lb % 2) * 64:(lb % 2) * 64 + S,
                                         (b // 2) * 768 + h * 96:
                                         (b // 2) * 768 + h * 96 + 96]
                                nc.tensor.matmul(
                                    po[:, h * 128:h * 128 + 100], vsl,
                                    attnT[(lb % 2) * 64:(lb % 2) * 64 + S,
                                          h * 400 + (lb // 2) * 100:
                                          h * 400 + (lb // 2) * 100 + 100],
                                    start=True, stop=True)
                            dst = _ap(oT[:, b * 100:], [[R, 8], [1, 100]])
                            nc.vector.tensor_copy(
                                out=dst, in_=_ap(po[:], [[128, 8], [1, 100]]))

        # ---- P3: attn_out + LN2 + FFN + LN3 -> h3T ----
        with pool("p3w") as p3w, pool("p3t") as p3t, \
             pool("p3f") as p3f, pool("p3s", bufs=2) as p3s, \
             pool("p3ps", bufs=4, space="PSUM") as p3ps, \
             pool("p3st", space="PSUM") as p3st, \
             pool("p3ab", space="PSUM") as p3ab:
            wao = p3w.tile([96, 8 * 768], BF16)
            nc.sync.dma_start(out=wao, in_=d["wao"])
            w1 = p3w.tile([128, 6 * 2048], BF16)
            nc.sync.dma_start(out=w1, in_=d["w1"])
            w2 = p3w.tile([128, 16 * 768], BF16)
            nc.sync.dma_start(out=w2, in_=d["w2"])

            def layer_norm_T(xin, gname, bname, yout):
                sq = p3f.tile([128, 6 * RC], BF16)
                nc.scalar.square(out=sq[:], in_=xin[:])
                s1 = p3st.tile([1, RC], F32)
                s2 = p3st.tile([1, RC], F32)
                for k in range(6):
                    nc.tensor.matmul(s1[:], ones_col[:],
                                     xin[:, k * RC:(k + 1) * RC],
                                     start=(k == 0), stop=(k == 5))
                for k in range(6):
                    nc.tensor.matmul(s2[:], ones_col[:],
                                     sq[:, k * RC:(k + 1) * RC],
                                     start=(k == 0), stop=(k == 5))
                mean = p3f.tile([1, RC], F32)
                nc.vector.tensor_scalar_mul(out=mean[:], in0=s1[:],
                                            scalar1=1.0 / D)
                var = p3f.tile([1, RC], F32)
                nc.vector.tensor_scalar_mul(out=var[:], in0=s2[:],
                                            scalar1=1.0 / D)
                msq = p3f.tile([1, RC], F32)
                nc.vector.tensor_tensor(out=msq[:], in0=mean[:], in1=mean[:],
                                        op=ALU.mult)
                nc.vector.tensor_tensor(out=var[:], in0=var[:], in1=msq[:],
                                        op=ALU.subtract)
                sd = p3f.tile([1, RC], F32)
                nc.scalar.activation(out=sd[:], in_=var[:], func=AF.Sqrt,
                                     bias=eps_t[:], scale=1.0)
                rstd = p3f.tile([1, RC], F32)
                nc.vector.reciprocal(out=rstd[:], in_=sd[:])
                nmr = p3f.tile([1, RC], F32)
                nc.vector.tensor_tensor(out=nmr[:], in0=mean[:], in1=rstd[:],
                                        op=ALU.mult)
                rstd_b = p3f.tile([1, RC], BF16)
                nc.vector.tensor_copy(out=rstd_b[:], in_=rstd[:])
                nmr_b = p3f.tile([1, RC], BF16)
                nc.vector.tensor_scalar_mul(out=nmr_b[:], in0=nmr[:], scalar1=-1.0)
                pa = p3ab.tile([128, RC], F32)
                nc.tensor.matmul(pa[:], ones_row[:], rstd_b[:],
                                 start=True, stop=True)
                pb = p3ab.tile([128, RC], F32)
                nc.tensor.matmul(pb[:], ones_row[:], nmr_b[:],
                                 start=True, stop=True)
                gv, bv = small[gname], small[bname]
                for k in range(6):
                    u = p3s.tile([128, RC], F32)
                    nc.vector.tensor_tensor(out=u[:],
                                            in0=xin[:, k * RC:(k + 1) * RC],
                                            in1=pa[:], op=ALU.mult)
                    if ln_triv:
                        nc.vector.tensor_tensor(out=yout(k), in0=u[:],
                                                in1=pb[:], op=ALU.add)
                    else:
                        nc.vector.tensor_tensor(out=u[:], in0=u[:], in1=pb[:],
                                                op=ALU.add)
                        nc.vector.tensor_scalar(out=yout(k), in0=u[:],
                                                scalar1=gv[:, k:k + 1],
                                                scalar2=bv[:, k:k + 1],
                                                op0=ALU.mult, op1=ALU.add)

            for c in range(NCHUNK):
                t2 = p3t.tile([128, 6 * RC], BF16)
                for m in range(6):
                    ps = p3ps.tile([128, RC], F32)
                    for kh in range(H):
                        nc.tensor.matmul(
                            ps[:],
                            wao[:, kh * 768 + m * 128:kh * 768 + m * 128 + 128],
                            oT[:, kh * R + c * RC:kh * R + (c + 1) * RC],
                            start=(kh == 0), stop=(kh == 7))
                    ta = p3s.tile([128, RC], BF16)
                    nc.scalar.activation(out=ta[:], in_=ps[:], func=AF.Identity,
                                         bias=small["bao"][:, m:m + 1], scale=1.0)
                    tg = small["tgt0"][:, m * 100:(m + 1) * 100]
                    nc.vector.tensor_tensor(out=t2[:, m * RC:(m + 1) * RC],
                                            in0=ta[:],
                                            in1=_ap(tg, [[0, 4], [1, 100]]),
                                            op=ALU.add)
                y2 = p3t.tile([128, 6 * RC], BF16)
                if ffn_triv:
                    # b1=b2=0 and trivial LN gains: relu is positive-
                    # homogeneous and LN3 is row-scale invariant, so LN2's
                    # rstd can be dropped entirely; center by mean only.
                    s1 = p3st.tile([1, RC], F32)
                    for k in range(6):
                        nc.tensor.matmul(s1[:], ones_col[:],
                                         t2[:, k * RC:(k + 1) * RC],
                                         start=(k == 0), stop=(k == 5))
                    nmean_b = p3f.tile([1, RC], BF16)
                    nc.vector.tensor_scalar_mul(out=nmean_b[:], in0=s1[:],
                                                scalar1=-1.0 / D)
                    pb = p3ab.tile([128, RC], F32)
                    nc.tensor.matmul(pb[:], ones_row[:], nmean_b[:],
                                     start=True, stop=True)
                    for k in range(6):
                        nc.vector.tensor_tensor(
                            out=y2[:, k * RC:(k + 1) * RC],
                            in0=t2[:, k * RC:(k + 1) * RC],
                            in1=pb[:], op=ALU.add)
                else:
                    layer_norm_T(t2, "ln2g", "ln2b",
                                 lambda k: y2[:, k * RC:(k + 1) * RC])
                ff1 = p3f.tile([128, 16 * RC], BF16)
                for mf in range(16):
                    ps = p3ps.tile([128, RC], F32)
                    for k in range(6):
                        nc.tensor.matmul(
                            ps[:],
                            w1[:, k * 2048 + mf * 128:k * 2048 + mf * 128 + 128],
                            y2[:, k * RC:(k + 1) * RC],
                            start=(k == 0), stop=(k == 5))
                    nc.scalar.activation(out=ff1[:, mf * RC:(mf + 1) * RC],
                                         in_=ps[:], func=AF.Relu,
                                         bias=small["b1"][:, mf:mf + 1],
                                         scale=1.0)
                t3 = p3t.tile([128, 6 * RC], BF16)
                for m in range(6):
                    ps = p3ps.tile([128, RC], F32)
                    for k in range(16):
                        nc.tensor.matmul(
                            ps[:],
                            w2[:, k * 768 + m * 128:k * 768 + m * 128 + 128],
                            ff1[:, k * RC:(k + 1) * RC],
                            start=(k == 0), stop=(k == 15))
                    tb = p3s.tile([128, RC], BF16)
                    nc.scalar.activation(out=tb[:], in_=ps[:], func=AF.Identity,
                                         bias=small["b2"][:, m:m + 1], scale=1.0)
                    nc.vector.tensor_tensor(out=t3[:, m * RC:(m + 1) * RC],
                                            in0=tb[:],
                                            in1=y2[:, m * RC:(m + 1) * RC],
                                            op=ALU.add)
                if ffn_triv:
                    # defer rstd3 to the GroupFC evacuation: center t3 only,
                    # stash rstd per row (scale commutes with h3 @ dup_g,
                    # dup_bias==0 guaranteed by the skip_dupb gate below)
                    sq = p3f.tile([128, 6 * RC], BF16)
                    nc.scalar.square(out=sq[:], in_=t3[:])
                    s1 = p3st.tile([1, RC], F32)
                    s2 = p3st.tile([1, RC], F32)
                    for k in range(6):
                        nc.tensor.matmul(s1[:], ones_col[:],
                                         t3[:, k * RC:(k + 1) * RC],
                                         start=(k == 0), stop=(k == 5))
                    for k in range(6):
                        nc.tensor.matmul(s2[:], ones_col[:],
                                         sq[:, k * RC:(k + 1) * RC],
                                         start=(k == 0), stop=(k == 5))
                    mean = p3f.tile([1, RC], F32)
                    nc.vector.tensor_scalar_mul(out=mean[:], in0=s1[:],
                                                scalar1=1.0 / D)
                    var = p3f.tile([1, RC], F32)
                    nc.vector.tensor_scalar_mul(out=var[:], in0=s2[:],
                                                scalar1=1.0 / D)
                    msq = p3f.tile([1, RC], F32)
                    nc.vector.tensor_tensor(out=msq[:], in0=mean[:],
                                            in1=mean[:], op=ALU.mult)
                    nc.vector.tensor_tensor(out=var[:], in0=var[:], in1=msq[:],
                                            op=ALU.subtract)
                    sd = p3f.tile([1, RC], F32)
                    nc.scalar.activation(out=sd[:], in_=var[:], func=AF.Sqrt,
                                         bias=eps_t[:], scale=1.0)
                    rstd = p3f.tile([1, RC], F32)
                    nc.vector.reciprocal(out=rstd[:], in_=sd[:])
                    nc.vector.tensor_copy(
                        out=rstd_all[:, c * RC:(c + 1) * RC], in_=rstd[:])
                    nmean_b = p3f.tile([1, RC], BF16)
                    nc.vector.tensor_scalar_mul(out=nmean_b[:], in0=s1[:],
                                                scalar1=-1.0 / D)
                    pb = p3ab.tile([128, RC], F32)
                    nc.tensor.matmul(pb[:], ones_row[:], nmean_b[:],
                                     start=True, stop=True)
                    for k in range(6):
                        nc.vector.tensor_tensor(
                            out=h3T[:, k * R + c * RC:k * R + (c + 1) * RC],
                            in0=t3[:, k * RC:(k + 1) * RC],
                            in1=pb[:], op=ALU.add)
                else:
                    layer_norm_T(t3, "ln3g", "ln3b",
                                 lambda k: h3T[:, k * R + c * RC:k * R + (c + 1) * RC])

        # ---- P4: GroupFC -> logitsT ----
        with pool("p4d", bufs=16) as p4d, pool("p4o") as p4o, \
             pool("p4rs_sb", bufs=2) as p4rs_sb, \
             pool("p4ps", bufs=2, space="PSUM") as p4ps, \
             pool("p4rs", bufs=2, space="PSUM") as p4rs:
            logitsT = p4o.tile([96, G * BL], F32)
            dupb = p4o.tile(list(d["dupb"].shape), BF16)
            nc.sync.dma_start(out=dupb, in_=d["dupb"])
            for g0 in range(0, G, 16):
                ng = min(16, G - g0)
                ps = p4ps.tile([96, 16 * BL], F32)
                for gi in range(ng):
                    g = g0 + gi
                    dup = p4d.tile([128, 6 * 96], BF16)
                    nc.sync.dma_start(out=dup, in_=d["dup"][g])
                    if not skip_dupb:
                        nc.tensor.matmul(ps[:, gi * BL:(gi + 1) * BL],
                                         dupb[:, g * 96:(g + 1) * 96],
                                         ones32[:], start=True, stop=False)
                    for k in range(6):
                        hsl = _ap(h3T[:, k * R + g:], [[100, BL]])
                        nc.tensor.matmul(ps[:, gi * BL:(gi + 1) * BL],
                                         dup[:, k * 96:(k + 1) * 96],
                                         hsl, start=(skip_dupb and k == 0),
                                         stop=(k == 5))
                if ffn_triv:
                    rs_ps = p4rs.tile([96, 16 * BL], F32)
                    rsl = rstd_all[:, g0:]
                    nc.tensor.matmul(
                        rs_ps[:, 0:ng * BL], ones_row[:, 0:96],
                        _ap(rsl, [[1, ng], [100, BL]]),
                        start=True, stop=True)
                    rs_sb = p4rs_sb.tile([96, 16 * BL], BF16)
                    nc.scalar.copy(out=rs_sb[:, 0:ng * BL],
                                   in_=rs_ps[:, 0:ng * BL])
                    nc.vector.tensor_tensor(
                        out=logitsT[:, g0 * BL:(g0 + ng) * BL],
                        in0=ps[:, 0:ng * BL], in1=rs_sb[:, 0:ng * BL],
                        op=ALU.mult)
                else:
                    nc.vector.tensor_copy(out=logitsT[:, g0 * BL:(g0 + ng) * BL],
                                          in_=ps[:, 0:ng * BL])
            nc.sync.dma_start(out=out_d, in_=logitsT[:])


_CACHE = {}


def kernel(**inputs):
    f32 = lambda k: np.asarray(inputs[k], np.float32)
    x = f32("x")
    w_qkv, b_qkv = f32("w_qkv"), f32("b_qkv")
    w_attn_out, b_attn_out = f32("w_attn_out"), f32("b_attn_out")

    # host constant folding for the batch-independent query path
    t = 2.0 * f32("query_embed")
    mu = t.mean(-1, keepdims=True)
    va = ((t - mu) ** 2).mean(-1, keepdims=True)
    tgt0 = (t - mu) / np.sqrt(va + EPS) * f32("ln1_g") + f32("ln1_b")
    q = (tgt0 @ w_qkv[:, :D] + b_qkv[:D]) / np.sqrt(float(HD))
    bk = b_qkv[D:2 * D]
    qbk = np.stack([q[:, h * HD:(h + 1) * HD] @ bk[h * HD:(h + 1) * HD]
                    for h in range(H)], axis=1)
    bv = b_qkv[2 * D:]
    bao_eff = b_attn_out + bv @ w_attn_out   # softmax rows sum to 1

    col6 = lambda a: np.ascontiguousarray(a.reshape(6, 128).T)
    feed = {
        "wemb": _bf(f32("w_embed").reshape(16, 128, 768).transpose(1, 0, 2)
                    .reshape(128, -1)),
        "be": col6(f32("b_embed")),
        "wk": _bf(w_qkv[:, D:2 * D].reshape(6, 128, 768).transpose(1, 0, 2)
                  .reshape(128, -1)),
        "wv": _bf(w_qkv[:, 2 * D:].reshape(6, 128, 768).transpose(1, 0, 2)
                  .reshape(128, -1)),
        "wao": _bf(w_attn_out.reshape(8, 96, 768).transpose(1, 0, 2)
                   .reshape(96, -1)),
        "bao": col6(bao_eff),
        "w1": _bf(f32("w1").reshape(6, 128, 2048).transpose(1, 0, 2)
                  .reshape(128, -1)),
        "b1": np.ascontiguousarray(f32("b1").reshape(16, 128).T),
        "w2": _bf(f32("w2").reshape(16, 128, 768).transpose(1, 0, 2)
                  .reshape(128, -1)),
        "b2": col6(f32("b2")),
        "qT": _bf(q.T.reshape(8, 96, 100).transpose(1, 0, 2).reshape(96, -1)),
        "qbk": np.ascontiguousarray(qbk.astype(np.float32)),
        "tgt0": _bf(tgt0.T.reshape(6, 128, 100).transpose(1, 0, 2)
                    .reshape(128, -1)),
        "ln2g": col6(f32("ln2_g")), "ln2b": col6(f32("ln2_b")),
        "ln3g": col6(f32("ln3_g")), "ln3b": col6(f32("ln3_b")),
        "dup": _bf(f32("dup_pool").reshape(G, 6, 128, 96).transpose(0, 2, 1, 3)
                   .reshape(G, 128, 6 * 96)),
        "dupb": _bf(f32("dup_bias").reshape(1, -1)),
    }

    skip_dupb = bool(np.all(f32("dup_bias") == 0.0))
    ln_triv = bool(np.all(f32("ln2_g") == 1.0) and np.all(f32("ln2_b") == 0.0)
                   and np.all(f32("ln3_g") == 1.0) and np.all(f32("ln3_b") == 0.0))
    ffn_triv = bool(ln_triv and np.all(f32("b1") == 0.0)
                    and np.all(f32("b2") == 0.0))
    key = ("nc", skip_dupb, ln_triv, ffn_triv)
    if key not in _CACHE:
        _CACHE[key] = build_program(skip_dupb, ln_triv, ffn_triv)
    nc = _CACHE[key]
    _CACHE["nc"] = nc

    # xr[core] axes: [c, col, k, p]; device wants [c, p, k, col]
    xr = x.reshape(NCORES, XCH, XCOLS, 16, 128)
    in_maps = []
    for core in range(NCORES):
        xT = xr[core].transpose(0, 3, 2, 1).reshape(XCH, 128, 16 * XCOLS)
        in_maps.append({**feed, "xT": _bf(xT)})

    _CACHE["in_maps"] = in_maps
    res = run_bass_kernel_spmd(nc, in_maps, list(range(NCORES)))
    outs = []
    for core in range(NCORES):
        lt = np.asarray(res.results[core]["logitsT"], np.float32)
        outs.append(lt.reshape(96, G, BL).transpose(2, 1, 0).reshape(BL, G * DF))
    return np.concatenate(outs, axis=0).astype(np.float32)



# revision 25
# speedup vs baseline: 2.4118x; 2.4118x over previous
"""Trainium2 Bass kernel for nn_MLDecoder (moe_routing).

Data-parallel over batch across 8 NeuronCores (32 batch rows/core, head params
replicated). Activations stay feature-major ("transposed"): C^T = W^T A^T via
matmul(out=C^T, lhsT=W(natural), rhs=A^T). Rows r = b*100+g (b-major). The
batch-independent query path (tgt0, q) is constant-folded on the host. All
matmuls bf16 with fp32 PSUM; LN stats via ones-matmuls; softmax without
max-subtraction (scores are O(1) for this head).
"""
import sys
sys.path.insert(0, "/opt/trn_rl_repo")

import numpy as np
import ml_dtypes

import concourse.bass as bass
from concourse import bacc
import concourse.tile as tile
import concourse.mybir as mybir
from concourse.bass import AP
from concourse.bass_utils import run_bass_kernel_spmd
from concourse.masks import make_identity

F32 = mybir.dt.float32
BF16 = mybir.dt.bfloat16
BF = ml_dtypes.bfloat16
AF = mybir.ActivationFunctionType
ALU = mybir.AluOpType
AX = mybir.AxisListType

B, S, C0 = 256, 49, 2048
D, F = 768, 2048
G, DF = 100, 96
H, HD = 8, 96
EPS = 1e-5
NCORES = 8
BL = B // NCORES          # 32 batch rows per core
R = BL * G                # 3200 rows (b,g) per core
RC = 400                  # row chunk = 4 b
NCHUNK = R // RC
XCH = 4                   # x col chunks (8 b each)
XCOLS = (BL // XCH) * S   # 392
PADS = 64                 # padded spatial stride
MCOLS = BL * PADS         # 2048 padded mem cols


def _bf(a):
    return np.ascontiguousarray(a.astype(BF))


def _ap(base, free_dims):
    """Replace the free dims of a (sliced) AP, keeping its partition dim."""
    return AP(tensor=base.tensor, offset=base.offset,
              ap=[base.ap[0]] + [list(fd) for fd in free_dims])


def build_program(skip_dupb=False, ln_triv=False, ffn_triv=False):
    nc = bacc.Bacc("TRN2", target_bir_lowering=False, debug=False,
                   num_devices=NCORES)
    d = {}

    def din(name, shape, dt):
        d[name] = nc.dram_tensor(name, list(shape), dt, kind="ExternalInput").ap()

    din("xT", (XCH, 128, 16 * XCOLS), BF16)
    din("wemb", (128, 16 * 768), BF16)
    din("be", (128, 6), F32)
    din("wk", (128, 6 * 768), BF16)
    din("wv", (128, 6 * 768), BF16)
    din("wao", (96, 8 * 768), BF16)
    din("bao", (128, 6), F32)
    din("w1", (128, 6 * 2048), BF16)
    din("b1", (128, 16), F32)
    din("w2", (128, 16 * 768), BF16)
    din("b2", (128, 6), F32)
    din("qT", (96, 8 * 100), BF16)
    din("qbk", (100, 8), F32)
    din("tgt0", (128, 6 * 100), BF16)
    din("ln2g", (128, 6), F32)
    din("ln2b", (128, 6), F32)
    din("ln3g", (128, 6), F32)
    din("ln3b", (128, 6), F32)
    din("dup", (100, 128, 6 * 96), BF16)
    din("dupb", (1, G * 96), BF16)
    out_d = nc.dram_tensor("logitsT", [96, G * BL], F32,
                           kind="ExternalOutput").ap()

    with tile.TileContext(nc) as tc:
        build_kernel(tc, d, out_d, skip_dupb, ln_triv, ffn_triv)
    nc.compile()
    return nc


def build_kernel(tc, d, out_d, skip_dupb=False, ln_triv=False, ffn_triv=False):
    nc = tc.nc

    def pool(name, bufs=1, space="SBUF"):
        return tc.tile_pool(name=name, bufs=bufs, space=space)

    with pool("resident") as res, pool("h3pool") as h3p, pool("oTpool") as oTp:
        ident = res.tile([128, 128], BF16)
        make_identity(nc, ident[:])
        ones_col = res.tile([128, 1], BF16)
        nc.vector.memset(ones_col[:], 1.0)
        ones_row = res.tile([1, 128], BF16)
        nc.vector.memset(ones_row[:], 1.0)
        ones32 = res.tile([1, BL], BF16)
        nc.vector.memset(ones32[:], 1.0)
        eps_t = res.tile([1, 1], F32)
        nc.vector.memset(eps_t[:], EPS)

        small = {}
        for name in ["be", "bao", "b1", "b2", "qT", "qbk", "tgt0",
                     "ln2g", "ln2b", "ln3g", "ln3b"]:
            t = res.tile(list(d[name].shape), d[name].dtype, tag=name)
            nc.gpsimd.dma_start(out=t, in_=d[name])
            small[name] = t

        h3T = h3p.tile([128, 6 * R], BF16)
        rstd_all = h3p.tile([1, R], BF16)
        oT = oTp.tile([96, 8 * R], BF16)

        with pool("memTpool") as memp:
            memT = memp.tile([128, 6 * MCOLS], BF16)

            # ---- P0: mem^T = relu(We^T x^T + be), written b-padded ----
            with pool("p0w") as p0w, pool("p0x", bufs=2) as p0x, \
                 pool("p0ps", bufs=3, space="PSUM") as p0ps:
                wemb = p0w.tile([128, 16 * 768], BF16)
                nc.sync.dma_start(out=wemb, in_=d["wemb"])
                for c in range(XCH):
                    xt = p0x.tile([128, 16 * XCOLS], BF16)
                    nc.sync.dma_start(out=xt, in_=d["xT"][c])
                    for m in range(6):
                        ps = p0ps.tile([128, XCOLS], F32)
                        for k in range(16):
                            nc.tensor.matmul(
                                ps[:],
                                wemb[:, k * 768 + m * 128:k * 768 + m * 128 + 128],
                                xt[:, k * XCOLS:(k + 1) * XCOLS],
                                start=(k == 0), stop=(k == 15))
                        dst = _ap(memT[:, m * MCOLS + c * 8 * PADS:],
                                  [[PADS, 8], [1, S]])
                        src = _ap(ps[:], [[S, 8], [1, S]])
                        nc.scalar.activation(out=dst, in_=src, func=AF.Relu,
                                             bias=small["be"][:, m:m + 1],
                                             scale=1.0)

            # ---- P1: K^T (head-major, b-padded) and V (rows padded) ----
            with pool("kvpool") as kvp:
                KT = kvp.tile([96, 8 * MCOLS], BF16)
                Vp = kvp.tile([128, 16 * 768], BF16)
                with pool("p1w") as p1w, \
                     pool("p1ps", bufs=3, space="PSUM") as p1ps:
                    wk = p1w.tile([128, 6 * 768], BF16)
                    nc.sync.dma_start(out=wk, in_=d["wk"])
                    wv = p1w.tile([128, 6 * 768], BF16)
                    nc.sync.dma_start(out=wv, in_=d["wv"])
                    for h in range(H):
                        for c in range(XCH):
                            ps = p1ps.tile([96, XCOLS], F32)
                            for k in range(6):
                                rhs = _ap(memT[:, k * MCOLS + c * 8 * PADS:],
                                          [[PADS, 8], [1, S]])
                                nc.tensor.matmul(
                                    ps[:],
                                    wk[:, k * 768 + h * 96:k * 768 + h * 96 + 96],
                                    rhs, start=(k == 0), stop=(k == 5))
                            dst = _ap(KT[:, h * MCOLS + c * 8 * PADS:],
                                      [[PADS, 8], [1, S]])
                            nc.vector.tensor_copy(
                                out=dst, in_=_ap(ps[:], [[S, 8], [1, S]]))
                    for t in range(16):
                        ps = p1ps.tile([128, 768], F32)
                        for sub in range(2):
                            n0, n1 = sub * 512, min(768, (sub + 1) * 512)
                            for k in range(6):
                                nc.tensor.matmul(
                                    ps[:, n0:n1],
                                    memT[:, k * MCOLS + t * 128:
                                         k * MCOLS + t * 128 + 128],
                                    wv[:, k * 768 + n0:k * 768 + n1],
                                    start=(k == 0), stop=(k == 5))
                        nc.vector.tensor_copy(out=Vp[:, t * 768:(t + 1) * 768],
                                              in_=ps[:])

                # ---- P2: attention ----
                with pool("p2a", bufs=2) as p2a, pool("p2s", bufs=3) as p2s, \
                     pool("p2ps", bufs=2, space="PSUM") as psc, \
                     pool("p2pt", bufs=2, space="PSUM") as pst, \
                     pool("p2po", bufs=2, space="PSUM") as pso:
                    for bg in range(4):
                        attnT = p2a.tile([128, 8 * 400], BF16)
                        for h in range(H):
                            ps = psc.tile([100, 8 * S], F32)
                            rhs = _ap(KT[:, h * MCOLS + bg * 8 * PADS:],
                                      [[PADS, 8], [1, S]])
                            nc.tensor.matmul(ps[:],
                                             small["qT"][:, h * 100:(h + 1) * 100],
                                             rhs, start=True, stop=True)
                            # exp into 64-padded slots (pads hold garbage,
                            # excluded by every later access pattern)
                            att = p2s.tile([100, 8 * PADS], BF16)
                            nc.scalar.activation(out=_ap(att[:], [[PADS, 8], [1, S]]),
                                                 in_=ps[:],
                                                 func=AF.Exp,
                                                 bias=small["qbk"][:, h:h + 1],
                                                 scale=1.0)
                            sums = p2s.tile([100, 8], F32)
                            nc.vector.reduce_sum(out=sums[:],
                                                 in_=_ap(att[:], [[PADS, 8], [1, S]]),
                                                 axis=AX.X)
                            inv = p2s.tile([100, 8], F32)
                            nc.vector.reciprocal(out=inv[:], in_=sums[:])
                            attn = p2s.tile([100, 8 * PADS], BF16)
                            nc.vector.tensor_tensor(
                                out=_ap(attn[:], [[PADS, 8], [1, S]]),
                                in0=_ap(att[:], [[PADS, 8], [1, S]]),
                                in1=_ap(inv[:], [[1, 8], [0, S]]),
                                op=ALU.mult)
                            for pr in range(4):
                                pt = pst.tile([128, 100], BF16)
                                nc.tensor.transpose(
                                    pt[:], attn[:, pr * 128:(pr + 1) * 128],
                                    ident[0:100, 0:100])
                                nc.vector.tensor_copy(
                                    out=attnT[:, h * 400 + pr * 100:
                                              h * 400 + pr * 100 + 100],
                                    in_=pt[:])
                        for lb in range(8):
                            b = bg * 8 + lb
                            po = pso.tile([96, 1024], F32)
                            for h in range(H):
                                vsl = Vp[(lb % 2) * 64:(lb % 2) * 64 + S,
                                         (b // 2) * 768 + h * 96:
                                         (b // 2) * 768 + h * 96 + 96]
                                nc.tensor.matmul(
                                    po[:, h * 128:h * 128 + 100], vsl,
                                    attnT[(lb % 2) * 64:(lb % 2) * 64 + S,
                                          h * 400 + (lb // 2) * 100:
                                          h * 400 + (lb // 2) * 100 + 100],
                                    start=True, stop=True)
                            dst = _ap(oT[:, b * 100:], [[R, 8], [1, 100]])
                            nc.vector.tensor_copy(
                                out=dst, in_=_ap(po[:], [[128, 8], [1, 100]]))

        # ---- P3: attn_out + LN2 + FFN + LN3 -> h3T ----
        with pool("p3w") as p3w, pool("p3t") as p3t, \
             pool("p3f") as p3f, pool("p3s", bufs=2) as p3s, \
             pool("p3ps", bufs=4, space="PSUM") as p3ps, \
             pool("p3st", space="PSUM") as p3st, \
             pool("p3ab", space="PSUM") as p3ab:
            wao = p3w.tile([96, 8 * 768], BF16)
            nc.sync.dma_start(out=wao, in_=d["wao"])
            w1 = p3w.tile([128, 6 * 2048], BF16)
            nc.sync.dma_start(out=w1, in_=d["w1"])
            w2 = p3w.tile([128, 16 * 768], BF16)
            nc.sync.dma_start(out=w2, in_=d["w2"])

            def layer_norm_T(xin, gname, bname, yout):
                sq = p3f.tile([128, 6 * RC], BF16)
                nc.scalar.square(out=sq[:], in_=xin[:])
                s1 = p3st.tile([1, RC], F32)
                s2 = p3st.tile([1, RC], F32)
                for k in range(6):
                    nc.tensor.matmul(s1[:], ones_col[:],
                                     xin[:, k * RC:(k + 1) * RC],
                                     start=(k == 0), stop=(k == 5))
                for k in range(6):
                    nc.tensor.matmul(s2[:], ones_col[:],
                                     sq[:, k * RC:(k + 1) * RC],
                                     start=(k == 0), stop=(k == 5))
                mean = p3f.tile([1, RC], F32)
                nc.vector.tensor_scalar_mul(out=mean[:], in0=s1[:],
                                            scalar1=1.0 / D)
                var = p3f.tile([1, RC], F32)
                nc.vector.tensor_scalar_mul(out=var[:], in0=s2[:],
                                            scalar1=1.0 / D)
                msq = p3f.tile([1, RC], F32)
                nc.vector.tensor_tensor(out=msq[:], in0=mean[:], in1=mean[:],
                                        op=ALU.mult)
                nc.vector.tensor_tensor(out=var[:], in0=var[:], in1=msq[:],
                                        op=ALU.subtract)
                sd = p3f.tile([1, RC], F32)
                nc.scalar.activation(out=sd[:], in_=var[:], func=AF.Sqrt,
                                     bias=eps_t[:], scale=1.0)
                rstd = p3f.tile([1, RC], F32)
                nc.vector.reciprocal(out=rstd[:], in_=sd[:])
                nmr = p3f.tile([1, RC], F32)
                nc.vector.tensor_tensor(out=nmr[:], in0=mean[:], in1=rstd[:],
                                        op=ALU.mult)
                rstd_b = p3f.tile([1, RC], BF16)
                nc.vector.tensor_copy(out=rstd_b[:], in_=rstd[:])
                nmr_b = p3f.tile([1, RC], BF16)
                nc.vector.tensor_scalar_mul(out=nmr_b[:], in0=nmr[:], scalar1=-1.0)
                pa = p3ab.tile([128, RC], F32)
                nc.tensor.matmul(pa[:], ones_row[:], rstd_b[:],
                                 start=True, stop=True)
                pb = p3ab.tile([128, RC], F32)
                nc.tensor.matmul(pb[:], ones_row[:], nmr_b[:],
                                 start=True, stop=True)
                gv, bv = small[gname], small[bname]
                for k in range(6):
                    u = p3s.tile([128, RC], F32)
                    nc.vector.tensor_tensor(out=u[:],
                                            in0=xin[:, k * RC:(k + 1) * RC],
                                            in1=pa[:], op=ALU.mult)
                    if ln_triv:
                        nc.vector.tensor_tensor(out=yout(k), in0=u[:],
                                                in1=pb[:], op=ALU.add)
                    else:
                        nc.vector.tensor_tensor(out=u[:], in0=u[:], in1=pb[:],
                                                op=ALU.add)
                        nc.vector.tensor_scalar(out=yout(k), in0=u[:],
                                                scalar1=gv[:, k:k + 1],
                                                scalar2=bv[:, k:k + 1],
                                                op0=ALU.mult, op1=ALU.add)

            for c in range(NCHUNK):
                t2 = p3t.tile([128, 6 * RC], BF16)
                for m in range(6):
                    ps = p3ps.tile([128, RC], F32)
                    for kh in range(H):
                        nc.tensor.matmul(
                            ps[:],
                            wao[:, kh * 768 + m * 128:kh * 768 + m * 128 + 128],
                            oT[:, kh * R + c * RC:kh * R + (c + 1) * RC],
                            start=(kh == 0), stop=(kh == 7))
                    ta = p3s.tile([128, RC], BF16)
                    nc.scalar.activation(out=ta[:], in_=ps[:], func=AF.Identity,
                                         bias=small["bao"][:, m:m + 1], scale=1.0)
                    tg = small["tgt0"][:, m * 100:(m + 1) * 100]
                    nc.vector.tensor_tensor(out=t2[:, m * RC:(m + 1) * RC],
                                            in0=ta[:],
                                            in1=_ap(tg, [[0, 4], [1, 100]]),
                                            op=ALU.add)
                y2 = p3t.tile([128, 6 * RC], BF16)
                if ffn_triv:
                    # b1=b2=0 and trivial LN gains: relu is positive-
                    # homogeneous and LN3 is row-scale invariant, so LN2's
                    # rstd can be dropped entirely; center by mean only.
                    s1 = p3st.tile([1, RC], F32)
                    for k in range(6):
                        nc.tensor.matmul(s1[:], ones_col[:],
                                         t2[:, k * RC:(k + 1) * RC],
                                         start=(k == 0), stop=(k == 5))
                    nmean_b = p3f.tile([1, RC], BF16)
                    nc.vector.tensor_scalar_mul(out=nmean_b[:], in0=s1[:],
                                                scalar1=-1.0 / D)
                    pb = p3ab.tile([128, RC], F32)
                    nc.tensor.matmul(pb[:], ones_row[:], nmean_b[:],
                                     start=True, stop=True)
                    for k in range(6):
                        nc.vector.tensor_tensor(
                            out=y2[:, k * RC:(k + 1) * RC],
                            in0=t2[:, k * RC:(k + 1) * RC],
                            in1=pb[:], op=ALU.add)
                else:
                    layer_norm_T(t2, "ln2g", "ln2b",
                                 lambda k: y2[:, k * RC:(k + 1) * RC])
                ff1 = p3f.tile([128, 16 * RC], BF16)
                for mf in range(16):
                    ps = p3ps.tile([128, RC], F32)
                    for k in range(6):
                        nc.tensor.matmul(
                            ps[:],
                            w1[:, k * 2048 + mf * 128:k * 2048 + mf * 128 + 128],
                            y2[:, k * RC:(k + 1) * RC],
                            start=(k == 0), stop=(k == 5))
                    nc.scalar.activation(out=ff1[:, mf * RC:(mf + 1) * RC],
                                         in_=ps[:], func=AF.Relu,
                                         bias=small["b1"][:, mf:mf + 1],
                                         scale=1.0)
                t3 = p3t.tile([128, 6 * RC], BF16)
                for m in range(6):
                    ps = p3ps.tile([128, RC], F32)
                    for k in range(16):
                        nc.tensor.matmul(
                            ps[:],
                            w2[:, k * 768 + m * 128:k * 768 + m * 128 + 128],
                            ff1[:, k * RC:(k + 1) * RC],
                            start=(k == 0), stop=(k == 15))
                    tb = p3s.tile([128, RC], BF16)
                    nc.scalar.activation(out=tb[:], in_=ps[:], func=AF.Identity,
                                         bias=small["b2"][:, m:m + 1], scale=1.0)
                    nc.vector.tensor_tensor(out=t3[:, m * RC:(m + 1) * RC],
                                            in0=tb[:],
                                            in1=y2[:, m * RC:(m + 1) * RC],
                                            op=ALU.add)
                if ffn_triv:
                    # defer rstd3 to the GroupFC evacuation: center t3 only,
                    # stash rstd per row (scale commutes with h3 @ dup_g,
                    # dup_bias==0 guaranteed by the skip_dupb gate below)
                    sq = p3f.tile([128, 6 * RC], BF16)
                    nc.scalar.square(out=sq[:], in_=t3[:])
                    s1 = p3st.tile([1, RC], F32)
                    s2 = p3st.tile([1, RC], F32)
                    for k in range(6):
                        nc.tensor.matmul(s1[:], ones_col[:],
                                         t3[:, k * RC:(k + 1) * RC],
                                         start=(k == 0), stop=(k == 5))
                    for k in range(6):
                        nc.tensor.matmul(s2[:], ones_col[:],
                                         sq[:, k * RC:(k + 1) * RC],
                                         start=(k == 0), stop=(k == 5))
                    mean = p3f.tile([1, RC], F32)
                    nc.vector.tensor_scalar_mul(out=mean[:], in0=s1[:],
                                                scalar1=1.0 / D)
                    var = p3f.tile([1, RC], F32)
                    nc.vector.tensor_scalar_mul(out=var[:], in0=s2[:],
                                                scalar1=1.0 / D)
                    msq = p3f.tile([1, RC], F32)
                    nc.vector.tensor_tensor(out=msq[:], in0=mean[:],
                                            in1=mean[:], op=ALU.mult)
                    nc.vector.tensor_tensor(out=var[:], in0=var[:], in1=msq[:],
                                            op=ALU.subtract)
                    sd = p3f.tile([1, RC], F32)
                    nc.scalar.activation(out=sd[:], in_=var[:], func=AF.Sqrt,
                                         bias=eps_t[:], scale=1.0)
                    rstd = p3f.tile([1, RC], F32)
                    nc.vector.reciprocal(out=rstd[:], in_=sd[:])
                    nc.vector.tensor_copy(
                        out=rstd_all[:, c * RC:(c + 1) * RC], in_=rstd[:])
                    nmean_b = p3f.tile([1, RC], BF16)
                    nc.vector.tensor_scalar_mul(out=nmean_b[:], in0=s1[:],
                                                scalar1=-1.0 / D)
                    pb = p3ab.tile([128, RC], F32)
                    nc.tensor.matmul(pb[:], ones_row[:], nmean_b[:],
                                     start=True, stop=True)
                    for k in range(6):
                        nc.vector.tensor_tensor(
                            out=h3T[:, k * R + c * RC:k * R + (c + 1) * RC],
                            in0=t3[:, k * RC:(k + 1) * RC],
                            in1=pb[:], op=ALU.add)
                else:
                    layer_norm_T(t3, "ln3g", "ln3b",
                                 lambda k: h3T[:, k * R + c * RC:k * R + (c + 1) * RC])

        # ---- P4: GroupFC -> logitsT ----
        with pool("p4d", bufs=16) as p4d, pool("p4o") as p4o, \
             pool("p4rs_sb", bufs=2) as p4rs_sb, \
             pool("p4ps", bufs=2, space="PSUM") as p4ps, \
             pool("p4rs", bufs=2, space="PSUM") as p4rs:
            logitsT = p4o.tile([96, G * BL], F32)
            dupb = p4o.tile(list(d["dupb"].shape), BF16)
            nc.sync.dma_start(out=dupb, in_=d["dupb"])
            for g0 in range(0, G, 16):
                ng = min(16, G - g0)
                ps = p4ps.tile([96, 16 * BL], F32)
                for gi in range(ng):
                    g = g0 + gi
                    dup = p4d.tile([128, 6 * 96], BF16)
                    nc.sync.dma_start(out=dup, in_=d["dup"][g])
                    if not skip_dupb:
                        nc.tensor.matmul(ps[:, gi * BL:(gi + 1) * BL],
                                         dupb[:, g * 96:(g + 1) * 96],
                                         ones32[:], start=True, stop=False)
                    for k in range(6):
                        hsl = _ap(h3T[:, k * R + g:], [[100, BL]])
                        nc.tensor.matmul(ps[:, gi * BL:(gi + 1) * BL],
                                         dup[:, k * 96:(k + 1) * 96],
                                         hsl, start=(skip_dupb and k == 0),
                                         stop=(k == 5))
                if ffn_triv:
                    rs_ps = p4rs.tile([96, 16 * BL], F32)
                    rsl = rstd_all[:, g0:]
                    nc.tensor.matmul(
                        rs_ps[:, 0:ng * BL], ones_row[:, 0:96],
                        _ap(rsl, [[1, ng], [100, BL]]),
                        start=True, stop=True)
                    rs_sb = p4rs_sb.tile([96, 16 * BL], BF16)
                    nc.scalar.copy(out=rs_sb[:, 0:ng * BL],
                                   in_=rs_ps[:, 0:ng * BL])
                    nc.vector.tensor_tensor(
                        out=logitsT[:, g0 * BL:(g0 + ng) * BL],
                        in0=ps[:, 0:ng * BL], in1=rs_sb[:, 0:ng * BL],
                        op=ALU.mult)
                else:
                    nc.vector.tensor_copy(out=logitsT[:, g0 * BL:(g0 + ng) * BL],
                                          in_=ps[:, 0:ng * BL])
            nc.sync.dma_start(out=out_d, in_=logitsT[:])


_CACHE = {}


# ======================================================================
# Fast path: fp8 DoubleRow pipeline.
#
# Row order is g-major: r = g*32 + b (3200 rows/core, chunks of RC=512).
# All heavy matmuls run fp8e4 DoubleRow (0.5 cyc/row, 2x contraction per
# instr). Precision is preserved by splitting the FFN around the
# batch-independent query path: z0 = y0@w1, ff10 = relu(z0), and
# t30 = y0 + ff10@w2 are computed exactly on the host and re-injected
# with bf16 "indicator" matmuls; the device only quantizes the small
# batch-dependent delta (ow = o@wao, std ~0.1). LN2 centering is folded
# into w1 as a rank-1 host correction; the residual -mean(ow) term is
# dropped exactly (it vanishes in LN3). LN3 centering is folded into
# GroupFC via per-group colsum(dup) rank-1 matmuls; rstd is applied at
# the logits evacuation.
#
# Scales (powers of two): weights x128; qT x64; oT fp8 x8; ow bf16
# x4096; owq fp8 x16; psum1 = 2048*z; v = 32*ff1; dff fp8 = 32*dff;
# psum2/t3s = 4096*t3.
# ======================================================================

FP8 = mybir.dt.float8e4
F8NP = ml_dtypes.float8_e4m3fn
DR = mybir.MatmulPerfMode.DoubleRow
RC = 512
NCH = (R + RC - 1) // RC          # 7 (6x512 + 128)


def _f8(a, scale=1.0):
    a = np.asarray(a, np.float32) * scale
    return np.ascontiguousarray(np.clip(a, -240.0, 240.0).astype(F8NP))


def _bf16(a, scale=1.0):
    return np.ascontiguousarray((np.asarray(a, np.float32) * scale).astype(BF))


def build_fast(skip_dupb=True):
    nc = bacc.Bacc("TRN2", target_bir_lowering=False, debug=False,
                   num_devices=NCORES)
    d = {}

    def din(name, shape, dt):
        d[name] = nc.dram_tensor(name, list(shape), dt, kind="ExternalInput").ap()

    din("xT", (XCH, 128, 16 * 392), FP8)
    din("wemb", (128, 16 * 768), FP8)
    din("wk", (128, 6 * 768), FP8)
    din("wv", (128, 6 * 768), FP8)
    din("wao", (96, 8 * 768), FP8)
    din("qT", (96, 8 * 100), FP8)
    din("qbk", (100, 8), F32)
    din("be", (128, 6), F32)
    din("baos", (128, 6), F32)
    din("w1c", (128, 6 * 2048), FP8)
    din("w2", (128, 16 * 768), FP8)
    din("z0s", (100, 16 * 128), BF16)
    din("ff10s", (128, 16 * 100), BF16)
    din("t30s", (100, 6 * 128), BF16)
    din("csum", (1, G * 96), BF16)
    if not skip_dupb:
        din("dupb_s", (96, 100), BF16)
    din("dup", (G * 128, 576), BF16)
    out_d = nc.dram_tensor("logitsT", [96, R], F32, kind="ExternalOutput").ap()

    with tile.TileContext(nc) as tc:
        fast_kernel(tc, d, out_d, skip_dupb)
    nc.compile()
    return nc


def fast_kernel(tc, d, out_d, skip_dupb=True):
    nc = tc.nc

    def pool(name, bufs=1, space="SBUF"):
        return tc.tile_pool(name=name, bufs=bufs, space=space)

    with pool("res") as res, pool("dupp", bufs=3) as dupp, \
         pool("csp", bufs=2) as csp, pool("oTp") as oTp:
        identb = res.tile([128, 128], BF16)
        make_identity(nc, identb[:])
        ones_col = res.tile([128, 1], BF16)
        nc.vector.memset(ones_col[:], 1.0)
        ones_row = res.tile([1, 128], BF16)
        nc.vector.memset(ones_row[:], 1.0)
        ones2f8 = res.tile([128, 32], FP8)
        nc.vector.memset(ones2f8[:], 1.0)
        eps2 = res.tile([1, 1], F32)
        nc.vector.memset(eps2[:], EPS * (2.0 ** 24))

        small = {}
        names = ["qT", "qbk", "be", "baos", "z0s", "ff10s", "t30s"]
        if not skip_dupb:
            names.append("dupb_s")
        for name in names:
            t = res.tile(list(d[name].shape), d[name].dtype, tag=name)
            nc.gpsimd.dma_start(out=t, in_=d[name])
            small[name] = t
        w1cq = res.tile([128, 6 * 2048], FP8)
        nc.scalar.dma_start(out=w1cq, in_=d["w1c"])
        w2q = res.tile([128, 16 * 768], FP8)
        nc.scalar.dma_start(out=w2q, in_=d["w2"])
        waoq = res.tile([96, 8 * 768], FP8)
        nc.scalar.dma_start(out=waoq, in_=d["wao"])

        oTq = oTp.tile([96, 8 * R], FP8)

        CWs = [min(RC, R - c * RC) for c in range(NCH)]
        dup_halves, csum_ts = {}, {}
        HALVES = []
        for blk in range(NCH):
            ngs = CWs[blk] // 32
            HALVES.append((blk, blk * 16, min(8, ngs)))
            if ngs > 8:
                HALVES.append((blk, blk * 16 + 8, ngs - 8))
        fetch_state = {"i": 0}

        def fetch_next_halves(n):
            for _ in range(n):
                i = fetch_state["i"]
                if i >= len(HALVES):
                    return
                fetch_state["i"] = i + 1
                blk, g0, ng = HALVES[i]
                bt = dupp.tile([128, 8 * 576], BF16, tag="dh")
                src = AP(tensor=d["dup"].tensor, offset=g0 * 128 * 576,
                         ap=[[576, 128], [128 * 576, ng], [1, 576]])
                nc.scalar.dma_start(out=_ap(bt[:], [[576, ng], [1, 576]]),
                                    in_=src)
                dup_halves[g0] = bt
                if g0 == blk * 16:
                    ngc = CWs[blk] // 32
                    ct = csp.tile([1, 16 * 96], BF16, tag="cs")
                    nc.scalar.dma_start(
                        out=ct[:, 0:ngc * 96],
                        in_=d["csum"][:, blk * 1536:blk * 1536 + ngc * 96])
                    csum_ts[blk] = ct

        # ---------------- P0 + P1: mem, K^T, V ----------------
        with pool("kvp") as kvp:
            KTq = kvp.tile([96, 8 * 2048], FP8)
            Vq = kvp.tile([128, 16 * 768], BF16)
            with pool("memp") as memp, pool("p0w") as p0w, \
                 pool("p0x", bufs=2) as p0x, \
                 pool("p0ps", bufs=2, space="PSUM") as p0ps, \
                 pool("p1k", bufs=2, space="PSUM") as p1k, \
                 pool("p1v", bufs=2, space="PSUM") as p1v:
                memq = memp.tile([128, 6 * 2048], FP8)
                wembq = p0w.tile([128, 16 * 768], FP8)
                nc.scalar.dma_start(out=wembq, in_=d["wemb"])
                wkq = p0w.tile([128, 6 * 768], FP8)
                nc.scalar.dma_start(out=wkq, in_=d["wk"])
                wvq = p0w.tile([128, 6 * 768], FP8)
                nc.scalar.dma_start(out=wvq, in_=d["wv"])
                fetch_next_halves(3)

                for c in range(XCH):
                    xq = p0x.tile([128, 16 * 392], FP8)
                    nc.sync.dma_start(out=xq, in_=d["xT"][c])
                    for m in range(6):
                        ps = p0ps.tile([128, 392], F32)
                        for kp in range(8):
                            nc.tensor.matmul(
                                ps[:],
                                _ap(wembq[:, 2 * kp * 768 + m * 128:],
                                    [[768, 2], [1, 128]]),
                                _ap(xq[:, 2 * kp * 392:], [[392, 2], [1, 392]]),
                                start=(kp == 0), stop=(kp == 7), perf_mode=DR)
                        nc.scalar.activation(
                            out=_ap(memq[:, m * 2048 + c * 512:],
                                    [[64, 8], [1, S]]),
                            in_=_ap(ps[:], [[S, 8], [1, S]]),
                            func=AF.Relu, bias=small["be"][:, m:m + 1],
                            scale=2.0 ** -7)

                # K^T (head-major, b-padded 64)
                for h in range(H):
                    for c in range(XCH):
                        ps = p1k.tile([96, 392], F32)
                        for kp in range(3):
                            nc.tensor.matmul(
                                ps[:],
                                _ap(wkq[:, 2 * kp * 768 + h * 96:],
                                    [[768, 2], [1, 96]]),
                                _ap(memq[:, 2 * kp * 2048 + c * 512:],
                                    [[2048, 2], [64, 8], [1, S]]),
                                start=(kp == 0), stop=(kp == 2), perf_mode=DR)
                        dst = _ap(KTq[:, h * 2048 + c * 512:], [[64, 8], [1, S]])
                        src = _ap(ps[:], [[S, 8], [1, S]])
                        if h % 2 == 0:
                            nc.scalar.activation(out=dst, in_=src, func=AF.Copy,
                                                 scale=2.0 ** -7)
                        else:
                            nc.vector.tensor_scalar_mul(out=dst, in0=src,
                                                        scalar1=2.0 ** -7)

                # V (rows b-pair padded on partitions)
                for t in range(16):
                    ps = p1v.tile([128, 768], F32)
                    for sub in range(2):
                        n0, n1 = sub * 512, min(768, (sub + 1) * 512)
                        for kp in range(3):
                            nc.tensor.matmul(
                                ps[:, n0:n1],
                                _ap(memq[:, 2 * kp * 2048 + t * 128:],
                                    [[2048, 2], [1, 128]]),
                                _ap(wvq[:, 2 * kp * 768 + n0:],
                                    [[768, 2], [1, n1 - n0]]),
                                start=(kp == 0), stop=(kp == 2), perf_mode=DR)
                    dst = Vq[:, t * 768:(t + 1) * 768]
                    if t % 2 == 0:
                        nc.vector.tensor_scalar_mul(out=dst, in0=ps[:],
                                                    scalar1=2.0 ** -7)
                    else:
                        nc.scalar.activation(out=dst, in_=ps[:], func=AF.Copy,
                                             scale=2.0 ** -7)

            # ---------------- P2: attention ----------------
            with pool("p2a", bufs=2) as p2a, pool("p2s", bufs=3) as p2s, \
                 pool("p2q", bufs=2) as p2q, \
                 pool("p2ps", bufs=2, space="PSUM") as psc, \
                 pool("p2pt", bufs=2, space="PSUM") as pst, \
                 pool("p2po", bufs=2, space="PSUM") as pso:
                for bg in range(4):
                    attnT = p2a.tile([128, 8 * 400], BF16)
                    hdat = []
                    for h in range(H):
                        ps = psc.tile([100, 392], F32)
                        nc.tensor.matmul(
                            ps[:], small["qT"][:, h * 100:(h + 1) * 100],
                            _ap(KTq[:, h * 2048 + bg * 512:], [[64, 8], [1, S]]),
                            start=True, stop=True)
                        att = p2s.tile([100, 8 * 64], BF16)
                        nc.scalar.activation(
                            out=_ap(att[:], [[64, 8], [1, S]]), in_=ps[:],
                            func=AF.Exp, bias=small["qbk"][:, h:h + 1],
                            scale=2.0 ** -6)
                        sums = p2s.tile([100, 8], F32)
                        nc.vector.reduce_sum(out=sums[:],
                                             in_=_ap(att[:], [[64, 8], [1, S]]),
                                             axis=AX.X)
                        inv = p2s.tile([100, 8], BF16)
                        with nc.allow_low_precision(reason="softmax inv bf16"):
                            nc.vector.reciprocal(out=inv[:], in_=sums[:])
                        attnq = p2q.tile([100, 8 * 64], BF16, tag=f"aq{h}")
                        nc.gpsimd.tensor_tensor(
                            out=_ap(attnq[:], [[64, 8], [1, S]]),
                            in0=_ap(att[:], [[64, 8], [1, S]]),
                            in1=_ap(inv[:], [[1, 8], [0, S]]), op=ALU.mult)
                        hdat.append(attnq)
                    for h in range(H):
                        pt = pst.tile([128, 400], BF16)
                        for pr in range(4):
                            nc.tensor.transpose(
                                pt[:, pr * 100:(pr + 1) * 100],
                                hdat[h][:, pr * 128:(pr + 1) * 128],
                                identb[0:100, 0:100])
                        nc.vector.tensor_copy(out=attnT[:, h * 400:(h + 1) * 400],
                                              in_=pt[:])
                    for lb in range(8):
                        b = bg * 8 + lb
                        p0_ = (lb % 2) * 64
                        po = pso.tile([96, 1024], F32)
                        for h in range(H):
                            nc.tensor.matmul(
                                po[:, h * 128:h * 128 + 100],
                                Vq[p0_:p0_ + S,
                                   (b // 2) * 768 + h * 96:(b // 2) * 768 + h * 96 + 96],
                                attnT[p0_:p0_ + S,
                                      h * 400 + (lb // 2) * 100:
                                      h * 400 + (lb // 2) * 100 + 100],
                                start=True, stop=True)
                        dst = _ap(oTq[:, b:], [[R, 8], [32, 100]])
                        src = _ap(po[:], [[128, 8], [1, 100]])
                        if lb % 2 == 0:
                            nc.scalar.activation(out=dst, in_=src, func=AF.Copy,
                                                 scale=8.0)
                        else:
                            nc.vector.tensor_scalar_mul(out=dst, in0=src,
                                                        scalar1=8.0)

        # ---------------- P3 + P4: pipelined chunks ----------------
        # Per iteration (software pipeline, stage-shifted):
        #   C: FFN2 + LN3 stats for chunk it-2
        #   D: GroupFC + logits for chunk it-3
        #   A: attn_out for chunk it
        #   B: FFN1 -> dff (relu+subtract fused via max trick) for chunk it-1
        with pool("ows", bufs=3) as owsp, pool("owq", bufs=2) as owqp, \
             pool("dffp", bufs=2) as dffp, \
             pool("t3p", bufs=2) as t3p, pool("sqp", bufs=2) as sqp, \
             pool("smp", bufs=2) as smp, pool("logp", bufs=2) as logp, \
             pool("pmm", bufs=4, space="PSUM") as pmm, \
             pool("pstat", bufs=1, space="PSUM") as pstat, \
             pool("pp4", bufs=1, space="PSUM") as pp4:
            ows_t, owq_t, dff_t, t3_t, rse_t, mng_t = {}, {}, {}, {}, {}, {}

            for it in range(NCH + 3):
                # --- stage C: FFN2 + LN3 stats for chunk it-2 ---
                if 2 <= it < NCH + 2:
                    c = it - 2
                    cw, c0 = CWs[c], c * RC
                    dff = dff_t[c]
                    t3c = t3p.tile([128, 6 * RC], BF16, tag="t3")
                    for m in range(6):
                        ps = pmm.tile([128, RC], F32)
                        for kp in range(8):
                            nc.tensor.matmul(
                                ps[:, 0:cw],
                                _ap(w2q[:, 2 * kp * 768 + m * 128:],
                                    [[768, 2], [1, 128]]),
                                _ap(dff[:, 2 * kp * RC:], [[RC, 2], [1, cw]]),
                                start=(kp == 0), stop=False, perf_mode=DR)
                        nc.tensor.matmul(ps[:, 0:cw],
                                         small["t30s"][:, m * 128:(m + 1) * 128],
                                         _ap(identb[0:100, c * 16:],
                                             [[1, cw // 32], [0, 32]]),
                                         start=False, stop=True)
                        nc.vector.tensor_tensor(
                            out=t3c[:, m * RC:m * RC + cw], in0=ps[:, 0:cw],
                            in1=ows_t[c][:, m * RC:m * RC + cw], op=ALU.add)
                    t3_t[c] = t3c
                    # LN3 stats: s1 (bf16 ones matmuls), s2 (fp8 DR over sq)
                    s1 = pstat.tile([1, RC], F32)
                    for k in range(6):
                        nc.tensor.matmul(s1[:, 0:cw], ones_col[:],
                                         t3c[:, k * RC:k * RC + cw],
                                         start=(k == 0), stop=(k == 5))
                    sqc = sqp.tile([128, 6 * RC], FP8, tag="sq")
                    nc.scalar.activation(out=_ap(sqc[:], [[RC, 6], [1, cw]]),
                                         in_=_ap(t3c[:], [[RC, 6], [1, cw]]),
                                         func=AF.Square, scale=2.0 ** -12)
                    s2 = pstat.tile([16, RC], F32)
                    for kp in range(3):
                        nc.tensor.matmul(
                            s2[:, 0:cw], _ap(ones2f8[:], [[16, 2], [1, 16]]),
                            _ap(sqc[:, 2 * kp * RC:], [[RC, 2], [1, cw]]),
                            start=(kp == 0), stop=(kp == 2), perf_mode=DR)
                    s1sb = smp.tile([1, RC], F32, tag="s1")
                    nc.vector.tensor_copy(out=s1sb[:, 0:cw], in_=s1[:, 0:cw])
                    s2sb = smp.tile([1, RC], F32, tag="s2")
                    nc.vector.tensor_copy(out=s2sb[:, 0:cw], in_=s2[0:1, 0:cw])
                    # smalls: rstd = 1/sqrt(var+eps), scaled 2^-12 into rse;
                    # mng = -4096*mean. ACT funcs (Identity/Sqrt) share the
                    # sqrt_and_others table with Square/Copy - no reloads.
                    K1 = 1.0 / (768.0 * 4096.0)
                    m2 = smp.tile([1, RC], F32, tag="m2")
                    nc.scalar.activation(out=m2[:, 0:cw], in_=s1sb[:, 0:cw],
                                         func=AF.Identity,
                                         scale=K1 * (2.0 ** 12))
                    mng = smp.tile([1, RC], BF16, tag="mng")
                    nc.scalar.activation(out=mng[:, 0:cw], in_=s1sb[:, 0:cw],
                                         func=AF.Identity,
                                         scale=-4096.0 * K1)
                    var2 = smp.tile([1, RC], F32, tag="var2")
                    nc.scalar.activation(out=var2[:, 0:cw], in_=s2sb[:, 0:cw],
                                         func=AF.Identity, bias=eps2[:],
                                         scale=(2.0 ** 24) / 768.0)
                    msq = smp.tile([1, RC], F32, tag="msq")
                    nc.vector.tensor_tensor(out=msq[:, 0:cw], in0=m2[:, 0:cw],
                                            in1=m2[:, 0:cw], op=ALU.mult)
                    nc.vector.tensor_tensor(out=var2[:, 0:cw], in0=var2[:, 0:cw],
                                            in1=msq[:, 0:cw], op=ALU.subtract)
                    sd = smp.tile([1, RC], F32, tag="sd")
                    nc.scalar.activation(out=sd[:, 0:cw], in_=var2[:, 0:cw],
                                         func=AF.Sqrt)
                    rse = smp.tile([1, RC], BF16, tag="rse")
                    with nc.allow_low_precision(reason="rstd bf16"):
                        nc.vector.reciprocal(out=rse[:, 0:cw], in_=sd[:, 0:cw])
                    rse_t[c], mng_t[c] = rse, mng

                # --- stage D: GroupFC + logits for chunk it-3 ---
                if 3 <= it:
                    c = it - 3
                    cw, c0 = CWs[c], c * RC
                    t3c, rse, mng = t3_t[c], rse_t[c], mng_t[c]
                    cst = csum_ts.pop(c)
                    ps4 = pp4.tile([96, RC], F32)
                    for gi in range(cw // 32):
                        dblk = dup_halves[c * 16 + (gi // 8) * 8]
                        gl = gi % 8
                        for k in range(6):
                            nc.tensor.matmul(
                                ps4[:, gi * 32:(gi + 1) * 32],
                                dblk[:, gl * 576 + k * 96:gl * 576 + (k + 1) * 96],
                                t3c[:, k * RC + gi * 32:k * RC + gi * 32 + 32],
                                start=(k == 0), stop=False)
                        nc.tensor.matmul(
                            ps4[:, gi * 32:(gi + 1) * 32],
                            cst[:, gi * 96:(gi + 1) * 96],
                            mng[:, gi * 32:(gi + 1) * 32],
                            start=False, stop=True)
                    rs_ps = pp4.tile([96, RC], F32)
                    nc.tensor.matmul(rs_ps[:, 0:cw], ones_row[:, 0:96],
                                     rse[:, 0:cw], start=True, stop=True)
                    rs_sb = logp.tile([96, RC], BF16, tag="rs")
                    nc.scalar.activation(out=rs_sb[:, 0:cw], in_=rs_ps[:, 0:cw],
                                         func=AF.Copy)
                    logc = logp.tile([96, RC], F32, tag="log")
                    nc.vector.tensor_tensor(out=logc[:, 0:cw], in0=ps4[:, 0:cw],
                                            in1=rs_sb[:, 0:cw], op=ALU.mult)
                    if not skip_dupb:
                        nc.vector.tensor_tensor(
                            out=logc[:, 0:cw], in0=logc[:, 0:cw],
                            in1=_ap(small["dupb_s"][:, c * 16:],
                                    [[1, cw // 32], [0, 32]]),
                            op=ALU.add)
                    nc.sync.dma_start(out=out_d[:, c0:c0 + cw],
                                      in_=logc[:, 0:cw])
                    dup_halves.pop(c * 16)
                    dup_halves.pop(c * 16 + 8, None)
                    fetch_next_halves(2)

                # --- stage A: attn_out for chunk it ---
                if it < NCH:
                    c = it
                    cw, c0 = CWs[c], c * RC
                    ows = owsp.tile([128, 6 * RC], BF16, tag="ows")
                    for m in range(6):
                        ps = pmm.tile([128, RC], F32)
                        for hp in range(4):
                            nc.tensor.matmul(
                                ps[:, 0:cw],
                                _ap(waoq[:, 2 * hp * 768 + m * 128:],
                                    [[768, 2], [1, 128]]),
                                _ap(oTq[:, 2 * hp * R + c0:], [[R, 2], [1, cw]]),
                                start=(hp == 0), stop=(hp == 3), perf_mode=DR)
                        nc.scalar.activation(out=ows[:, m * RC:m * RC + cw],
                                             in_=ps[:, 0:cw], func=AF.Identity,
                                             bias=small["baos"][:, m:m + 1],
                                             scale=4.0)
                    owq = owqp.tile([128, 6 * RC], FP8, tag="owq")
                    nc.gpsimd.tensor_scalar_mul(
                        out=_ap(owq[:], [[RC, 6], [1, cw]]),
                        in0=_ap(ows[:], [[RC, 6], [1, cw]]), scalar1=2.0 ** -8)
                    ows_t[c], owq_t[c] = ows, owq

                # --- stage B: FFN1 -> dff for chunk it-1 ---
                # psum1 = 64*(dz + min(z0,0)); dff = max(psum1, -64*ff10)
                # equals 64*(relu(z0+dz) - relu(z0)) exactly.
                if 1 <= it < NCH + 1:
                    c = it - 1
                    cw, c0 = CWs[c], c * RC
                    owq = owq_t[c]
                    dff = dffp.tile([128, 16 * RC], FP8, tag="dff")
                    for mf in range(16):
                        ps = pmm.tile([128, RC], F32)
                        for kp in range(3):
                            nc.tensor.matmul(
                                ps[:, 0:cw],
                                _ap(w1cq[:, 2 * kp * 2048 + mf * 128:],
                                    [[2048, 2], [1, 128]]),
                                _ap(owq[:, 2 * kp * RC:], [[RC, 2], [1, cw]]),
                                start=(kp == 0), stop=False, perf_mode=DR)
                        nc.tensor.matmul(
                            ps[:, 0:cw],
                            small["z0s"][:, mf * 128:(mf + 1) * 128],
                            _ap(identb[0:100, c * 16:],
                                [[1, cw // 32], [0, 32]]),
                            start=False, stop=True)
                        nc.vector.tensor_tensor(
                            out=dff[:, mf * RC:mf * RC + cw], in0=ps[:, 0:cw],
                            in1=_ap(small["ff10s"][:, mf * 100 + c * 16:],
                                    [[1, cw // 32], [0, 32]]),
                            op=ALU.max)
                    dff_t[c] = dff


def _prep_fast(inputs, skip_dupb):
    f32 = lambda k: np.asarray(inputs[k], np.float32)
    x = f32("x")
    w_qkv, b_qkv = f32("w_qkv"), f32("b_qkv")
    w_attn_out, b_attn_out = f32("w_attn_out"), f32("b_attn_out")
    w1, w2, dup = f32("w1"), f32("w2"), f32("dup_pool")

    # batch-independent query path (host, exact)
    t = 2.0 * f32("query_embed")
    mu = t.mean(-1, keepdims=True)
    va = ((t - mu) ** 2).mean(-1, keepdims=True)
    tgt0 = (t - mu) / np.sqrt(va + EPS) * f32("ln1_g") + f32("ln1_b")
    q = (tgt0 @ w_qkv[:, :D] + b_qkv[:D]) / np.sqrt(float(HD))
    bk = b_qkv[D:2 * D]
    qbk = np.stack([q[:, h * HD:(h + 1) * HD] @ bk[h * HD:(h + 1) * HD]
                    for h in range(H)], axis=1)
    bv = b_qkv[2 * D:]
    bao_eff = b_attn_out + bv @ w_attn_out

    y0 = tgt0 - tgt0.mean(-1, keepdims=True)
    w1c = w1 - np.ones((D, 1), np.float32) @ (w1.sum(0, keepdims=True) / D)
    z0 = y0 @ w1
    ff10 = np.maximum(z0, 0.0)
    t30 = y0 + ff10 @ w2
    csum = dup.sum(1)                                    # [G, 96]

    col6 = lambda a: np.ascontiguousarray(a.reshape(6, 128).T.astype(np.float32))
    rr = np.arange(R)
    feed = {
        "wemb": _f8(f32("w_embed").reshape(16, 128, 768).transpose(1, 0, 2)
                    .reshape(128, -1), 128.0),
        "wk": _f8(w_qkv[:, D:2 * D].reshape(6, 128, 768).transpose(1, 0, 2)
                  .reshape(128, -1), 128.0),
        "wv": _f8(w_qkv[:, 2 * D:].reshape(6, 128, 768).transpose(1, 0, 2)
                  .reshape(128, -1), 128.0),
        "wao": _f8(w_attn_out.reshape(8, 96, 768).transpose(1, 0, 2)
                   .reshape(96, -1), 128.0),
        "qT": _f8(q.T.reshape(8, 96, 100).transpose(1, 0, 2).reshape(96, -1),
                  64.0),
        "qbk": np.ascontiguousarray(qbk.astype(np.float32)),
        "be": col6(f32("b_embed")),
        "baos": col6(bao_eff * 4096.0),
        "w1c": _f8(w1c.reshape(6, 128, 2048).transpose(1, 0, 2)
                   .reshape(128, -1), 4.0),
        "w2": _f8(w2.reshape(16, 128, 768).transpose(1, 0, 2)
                  .reshape(128, -1), 64.0),
        "z0s": _bf16(np.minimum(z0, 0.0).reshape(100, 16 * 128), 64.0),
        "ff10s": _bf16(-ff10.reshape(100, 16, 128).transpose(2, 1, 0)
                       .reshape(128, -1), 64.0),
        "t30s": _bf16(t30.reshape(100, 6 * 128), 4096.0),
        "ind": _bf16((rr[None, :] // BL) == np.arange(G)[:, None]),
        "csum": _bf16(csum.reshape(1, -1)),
        "dup": _bf16(dup.reshape(G, 6, 128, 96).transpose(0, 2, 1, 3)
                     .reshape(G * 128, 6 * 96)),
    }
    if not skip_dupb:
        feed["dupb_s"] = _bf16(f32("dup_bias").reshape(100, 96).T)

    xr = x.reshape(NCORES, XCH, 8, S, 16, 128)
    in_maps = []
    for core in range(NCORES):
        xT = xr[core].transpose(0, 4, 3, 1, 2).reshape(XCH, 128, 16 * 392)
        in_maps.append({**feed, "xT": _f8(xT)})
    return in_maps


def kernel(**inputs):
    f32 = lambda k: np.asarray(inputs[k], np.float32)
    fast_ok = bool(
        np.all(f32("ln2_g") == 1.0) and np.all(f32("ln2_b") == 0.0)
        and np.all(f32("ln3_g") == 1.0) and np.all(f32("ln3_b") == 0.0)
        and np.all(f32("b1") == 0.0) and np.all(f32("b2") == 0.0))
    if fast_ok:
        skip_dupb = bool(np.all(f32("dup_bias") == 0.0))
        key = ("fast", skip_dupb)
        if key not in _CACHE:
            _CACHE[key] = build_fast(skip_dupb)
        nc = _CACHE[key]
        _CACHE["nc"] = nc
        in_maps = _prep_fast(inputs, skip_dupb)
        _CACHE["in_maps"] = in_maps
        res = run_bass_kernel_spmd(nc, in_maps, list(range(NCORES)))
        outs = []
        for core in range(NCORES):
            lt = np.asarray(res.results[core]["logitsT"], np.float32)
            outs.append(lt.reshape(96, G, BL).transpose(2, 1, 0)
                        .reshape(BL, G * DF))
        return np.concatenate(outs, axis=0).astype(np.float32)

    x = f32("x")
    w_qkv, b_qkv = f32("w_qkv"), f32("b_qkv")
    w_attn_out, b_attn_out = f32("w_attn_out"), f32("b_attn_out")

    # host constant folding for the batch-independent query path
    t = 2.0 * f32("query_embed")
    mu = t.mean(-1, keepdims=True)
    va = ((t - mu) ** 2).mean(-1, keepdims=True)
    tgt0 = (t - mu) / np.sqrt(va + EPS) * f32("ln1_g") + f32("ln1_b")
    q = (tgt0 @ w_qkv[:, :D] + b_qkv[:D]) / np.sqrt(float(HD))
    bk = b_qkv[D:2 * D]
    qbk = np.stack([q[:, h * HD:(h + 1) * HD] @ bk[h * HD:(h + 1) * HD]
                    for h in range(H)], axis=1)
    bv = b_qkv[2 * D:]
    bao_eff = b_attn_out + bv @ w_attn_out   # softmax rows sum to 1

    col6 = lambda a: np.ascontiguousarray(a.reshape(6, 128).T)
    feed = {
        "wemb": _bf(f32("w_embed").reshape(16, 128, 768).transpose(1, 0, 2)
                    .reshape(128, -1)),
        "be": col6(f32("b_embed")),
        "wk": _bf(w_qkv[:, D:2 * D].reshape(6, 128, 768).transpose(1, 0, 2)
                  .reshape(128, -1)),
        "wv": _bf(w_qkv[:, 2 * D:].reshape(6, 128, 768).transpose(1, 0, 2)
                  .reshape(128, -1)),
        "wao": _bf(w_attn_out.reshape(8, 96, 768).transpose(1, 0, 2)
                   .reshape(96, -1)),
        "bao": col6(bao_eff),
        "w1": _bf(f32("w1").reshape(6, 128, 2048).transpose(1, 0, 2)
                  .reshape(128, -1)),
        "b1": np.ascontiguousarray(f32("b1").reshape(16, 128).T),
        "w2": _bf(f32("w2").reshape(16, 128, 768).transpose(1, 0, 2)
                  .reshape(128, -1)),
        "b2": col6(f32("b2")),
        "qT": _bf(q.T.reshape(8, 96, 100).transpose(1, 0, 2).reshape(96, -1)),
        "qbk": np.ascontiguousarray(qbk.astype(np.float32)),
        "tgt0": _bf(tgt0.T.reshape(6, 128, 100).transpose(1, 0, 2)
                    .reshape(128, -1)),
        "ln2g": col6(f32("ln2_g")), "ln2b": col6(f32("ln2_b")),
        "ln3g": col6(f32("ln3_g")), "ln3b": col6(f32("ln3_b")),
        "dup": _bf(f32("dup_pool").reshape(G, 6, 128, 96).transpose(0, 2, 1, 3)
                   .reshape(G, 128, 6 * 96)),
        "dupb": _bf(f32("dup_bias").reshape(1, -1)),
    }

    skip_dupb = bool(np.all(f32("dup_bias") == 0.0))
    ln_triv = bool(np.all(f32("ln2_g") == 1.0) and np.all(f32("ln2_b") == 0.0)
                   and np.all(f32("ln3_g") == 1.0) and np.all(f32("ln3_b") == 0.0))
    ffn_triv = bool(ln_triv and np.all(f32("b1") == 0.0)
                    and np.all(f32("b2") == 0.0))
    key = ("nc", skip_dupb, ln_triv, ffn_triv)
    if key not in _CACHE:
        _CACHE[key] = build_program(skip_dupb, ln_triv, ffn_triv)
    nc = _CACHE[key]
    _CACHE["nc"] = nc

    # xr[core] axes: [c, col, k, p]; device wants [c, p, k, col]
    xr = x.reshape(NCORES, XCH, XCOLS, 16, 128)
    in_maps = []
    for core in range(NCORES):
        xT = xr[core].transpose(0, 3, 2, 1).reshape(XCH, 128, 16 * XCOLS)
        in_maps.append({**feed, "xT": _bf(xT)})

    _CACHE["in_maps"] = in_maps
    res = run_bass_kernel_spmd(nc, in_maps, list(range(NCORES)))
    outs = []
    for core in range(NCORES):
        lt = np.asarray(res.results[core]["logitsT"], np.float32)
        outs.append(lt.reshape(96, G, BL).transpose(2, 1, 0).reshape(BL, G * DF))
    return np.concatenate(outs, axis=0).astype(np.float32)



# revision 34
# speedup vs baseline: 2.6303x; 1.0906x over previous
"""Trainium2 Bass kernel for nn_MLDecoder (moe_routing).

Data-parallel over batch across 8 NeuronCores (32 batch rows/core, head params
replicated). Activations stay feature-major ("transposed"): C^T = W^T A^T via
matmul(out=C^T, lhsT=W(natural), rhs=A^T). Rows r = b*100+g (b-major). The
batch-independent query path (tgt0, q) is constant-folded on the host. All
matmuls bf16 with fp32 PSUM; LN stats via ones-matmuls; softmax without
max-subtraction (scores are O(1) for this head).
"""
import sys
sys.path.insert(0, "/opt/trn_rl_repo")

import numpy as np
import ml_dtypes

import concourse.bass as bass
from concourse import bacc
import concourse.tile as tile
import concourse.mybir as mybir
from concourse.bass import AP
from concourse.bass_utils import run_bass_kernel_spmd
from concourse.masks import make_identity

F32 = mybir.dt.float32
BF16 = mybir.dt.bfloat16
BF = ml_dtypes.bfloat16
AF = mybir.ActivationFunctionType
ALU = mybir.AluOpType
AX = mybir.AxisListType

B, S, C0 = 256, 49, 2048
D, F = 768, 2048
G, DF = 100, 96
H, HD = 8, 96
EPS = 1e-5
NCORES = 8
BL = B // NCORES          # 32 batch rows per core
R = BL * G                # 3200 rows (b,g) per core
RC = 400                  # row chunk = 4 b
NCHUNK = R // RC
XCH = 4                   # x col chunks (8 b each)
XCOLS = (BL // XCH) * S   # 392
PADS = 64                 # padded spatial stride
MCOLS = BL * PADS         # 2048 padded mem cols


def _bf(a):
    return np.ascontiguousarray(a.astype(BF))


def _ap(base, free_dims):
    """Replace the free dims of a (sliced) AP, keeping its partition dim."""
    return AP(tensor=base.tensor, offset=base.offset,
              ap=[base.ap[0]] + [list(fd) for fd in free_dims])


def build_program(skip_dupb=False, ln_triv=False, ffn_triv=False):
    nc = bacc.Bacc("TRN2", target_bir_lowering=False, debug=False,
                   num_devices=NCORES)
    d = {}

    def din(name, shape, dt):
        d[name] = nc.dram_tensor(name, list(shape), dt, kind="ExternalInput").ap()

    din("xT", (XCH, 128, 16 * XCOLS), BF16)
    din("wemb", (128, 16 * 768), BF16)
    din("be", (128, 6), F32)
    din("wk", (128, 6 * 768), BF16)
    din("wv", (128, 6 * 768), BF16)
    din("wao", (96, 8 * 768), BF16)
    din("bao", (128, 6), F32)
    din("w1", (128, 6 * 2048), BF16)
    din("b1", (128, 16), F32)
    din("w2", (128, 16 * 768), BF16)
    din("b2", (128, 6), F32)
    din("qT", (96, 8 * 100), BF16)
    din("qbk", (100, 8), F32)
    din("tgt0", (128, 6 * 100), BF16)
    din("ln2g", (128, 6), F32)
    din("ln2b", (128, 6), F32)
    din("ln3g", (128, 6), F32)
    din("ln3b", (128, 6), F32)
    din("dup", (100, 128, 6 * 96), BF16)
    din("dupb", (1, G * 96), BF16)
    out_d = nc.dram_tensor("logitsT", [96, G * BL], F32,
                           kind="ExternalOutput").ap()

    with tile.TileContext(nc) as tc:
        build_kernel(tc, d, out_d, skip_dupb, ln_triv, ffn_triv)
    nc.compile()
    return nc


def build_kernel(tc, d, out_d, skip_dupb=False, ln_triv=False, ffn_triv=False):
    nc = tc.nc

    def pool(name, bufs=1, space="SBUF"):
        return tc.tile_pool(name=name, bufs=bufs, space=space)

    with pool("resident") as res, pool("h3pool") as h3p, pool("oTpool") as oTp:
        ident = res.tile([128, 128], BF16)
        make_identity(nc, ident[:])
        ones_col = res.tile([128, 1], BF16)
        nc.vector.memset(ones_col[:], 1.0)
        ones_row = res.tile([1, 128], BF16)
        nc.vector.memset(ones_row[:], 1.0)
        ones32 = res.tile([1, BL], BF16)
        nc.vector.memset(ones32[:], 1.0)
        eps_t = res.tile([1, 1], F32)
        nc.vector.memset(eps_t[:], EPS)

        small = {}
        for name in ["be", "bao", "b1", "b2", "qT", "qbk", "tgt0",
                     "ln2g", "ln2b", "ln3g", "ln3b"]:
            t = res.tile(list(d[name].shape), d[name].dtype, tag=name)
            nc.gpsimd.dma_start(out=t, in_=d[name])
            small[name] = t

        h3T = h3p.tile([128, 6 * R], BF16)
        rstd_all = h3p.tile([1, R], BF16)
        oT = oTp.tile([96, 8 * R], BF16)

        with pool("memTpool") as memp:
            memT = memp.tile([128, 6 * MCOLS], BF16)

            # ---- P0: mem^T = relu(We^T x^T + be), written b-padded ----
            with pool("p0w") as p0w, pool("p0x", bufs=2) as p0x, \
                 pool("p0ps", bufs=3, space="PSUM") as p0ps:
                wemb = p0w.tile([128, 16 * 768], BF16)
                nc.sync.dma_start(out=wemb, in_=d["wemb"])
                for c in range(XCH):
                    xt = p0x.tile([128, 16 * XCOLS], BF16)
                    nc.sync.dma_start(out=xt, in_=d["xT"][c])
                    for m in range(6):
                        ps = p0ps.tile([128, XCOLS], F32)
                        for k in range(16):
                            nc.tensor.matmul(
                                ps[:],
                                wemb[:, k * 768 + m * 128:k * 768 + m * 128 + 128],
                                xt[:, k * XCOLS:(k + 1) * XCOLS],
                                start=(k == 0), stop=(k == 15))
                        dst = _ap(memT[:, m * MCOLS + c * 8 * PADS:],
                                  [[PADS, 8], [1, S]])
                        src = _ap(ps[:], [[S, 8], [1, S]])
                        nc.scalar.activation(out=dst, in_=src, func=AF.Relu,
                                             bias=small["be"][:, m:m + 1],
                                             scale=1.0)

            # ---- P1: K^T (head-major, b-padded) and V (rows padded) ----
            with pool("kvpool") as kvp:
                KT = kvp.tile([96, 8 * MCOLS], BF16)
                Vp = kvp.tile([128, 16 * 768], BF16)
                with pool("p1w") as p1w, \
                     pool("p1ps", bufs=3, space="PSUM") as p1ps:
                    wk = p1w.tile([128, 6 * 768], BF16)
                    nc.sync.dma_start(out=wk, in_=d["wk"])
                    wv = p1w.tile([128, 6 * 768], BF16)
                    nc.sync.dma_start(out=wv, in_=d["wv"])
                    for h in range(H):
                        for c in range(XCH):
                            ps = p1ps.tile([96, XCOLS], F32)
                            for k in range(6):
                                rhs = _ap(memT[:, k * MCOLS + c * 8 * PADS:],
                                          [[PADS, 8], [1, S]])
                                nc.tensor.matmul(
                                    ps[:],
                                    wk[:, k * 768 + h * 96:k * 768 + h * 96 + 96],
                                    rhs, start=(k == 0), stop=(k == 5))
                            dst = _ap(KT[:, h * MCOLS + c * 8 * PADS:],
                                      [[PADS, 8], [1, S]])
                            nc.vector.tensor_copy(
                                out=dst, in_=_ap(ps[:], [[S, 8], [1, S]]))
                    for t in range(16):
                        ps = p1ps.tile([128, 768], F32)
                        for sub in range(2):
                            n0, n1 = sub * 512, min(768, (sub + 1) * 512)
                            for k in range(6):
                                nc.tensor.matmul(
                                    ps[:, n0:n1],
                                    memT[:, k * MCOLS + t * 128:
                                         k * MCOLS + t * 128 + 128],
                                    wv[:, k * 768 + n0:k * 768 + n1],
                                    start=(k == 0), stop=(k == 5))
                        nc.vector.tensor_copy(out=Vp[:, t * 768:(t + 1) * 768],
                                              in_=ps[:])

                # ---- P2: attention ----
                with pool("p2a", bufs=2) as p2a, pool("p2s", bufs=3) as p2s, \
                     pool("p2ps", bufs=2, space="PSUM") as psc, \
                     pool("p2pt", bufs=2, space="PSUM") as pst, \
                     pool("p2po", bufs=2, space="PSUM") as pso:
                    for bg in range(4):
                        attnT = p2a.tile([128, 8 * 400], BF16)
                        for h in range(H):
                            ps = psc.tile([100, 8 * S], F32)
                            rhs = _ap(KT[:, h * MCOLS + bg * 8 * PADS:],
                                      [[PADS, 8], [1, S]])
                            nc.tensor.matmul(ps[:],
                                             small["qT"][:, h * 100:(h + 1) * 100],
                                             rhs, start=True, stop=True)
                            # exp into 64-padded slots (pads hold garbage,
                            # excluded by every later access pattern)
                            att = p2s.tile([100, 8 * PADS], BF16)
                            nc.scalar.activation(out=_ap(att[:], [[PADS, 8], [1, S]]),
                                                 in_=ps[:],
                                                 func=AF.Exp,
                                                 bias=small["qbk"][:, h:h + 1],
                                                 scale=1.0)
                            sums = p2s.tile([100, 8], F32)
                            nc.vector.reduce_sum(out=sums[:],
                                                 in_=_ap(att[:], [[PADS, 8], [1, S]]),
                                                 axis=AX.X)
                            inv = p2s.tile([100, 8], F32)
                            nc.vector.reciprocal(out=inv[:], in_=sums[:])
                            attn = p2s.tile([100, 8 * PADS], BF16)
                            nc.vector.tensor_tensor(
                                out=_ap(attn[:], [[PADS, 8], [1, S]]),
                                in0=_ap(att[:], [[PADS, 8], [1, S]]),
                                in1=_ap(inv[:], [[1, 8], [0, S]]),
                                op=ALU.mult)
                            for pr in range(4):
                                pt = pst.tile([128, 100], BF16)
                                nc.tensor.transpose(
                                    pt[:], attn[:, pr * 128:(pr + 1) * 128],
                                    ident[0:100, 0:100])
                                nc.vector.tensor_copy(
                                    out=attnT[:, h * 400 + pr * 100:
                                              h * 400 + pr * 100 + 100],
                                    in_=pt[:])
                        for lb in range(8):
                            b = bg * 8 + lb
                            po = pso.tile([96, 1024], F32)
                            for h in range(H):
                                vsl = Vp[(lb % 2) * 64:(lb % 2) * 64 + S,
                                         (b // 2) * 768 + h * 96:
                                         (b // 2) * 768 + h * 96 + 96]
                                nc.tensor.matmul(
                                    po[:, h * 128:h * 128 + 100], vsl,
                                    attnT[(lb % 2) * 64:(lb % 2) * 64 + S,
                                          h * 400 + (lb // 2) * 100:
                                          h * 400 + (lb // 2) * 100 + 100],
                                    start=True, stop=True)
                            dst = _ap(oT[:, b * 100:], [[R, 8], [1, 100]])
                            nc.vector.tensor_copy(
                                out=dst, in_=_ap(po[:], [[128, 8], [1, 100]]))

        # ---- P3: attn_out + LN2 + FFN + LN3 -> h3T ----
        with pool("p3w") as p3w, pool("p3t") as p3t, \
             pool("p3f") as p3f, pool("p3s", bufs=2) as p3s, \
             pool("p3ps", bufs=4, space="PSUM") as p3ps, \
             pool("p3st", space="PSUM") as p3st, \
             pool("p3ab", space="PSUM") as p3ab:
            wao = p3w.tile([96, 8 * 768], BF16)
            nc.sync.dma_start(out=wao, in_=d["wao"])
            w1 = p3w.tile([128, 6 * 2048], BF16)
            nc.sync.dma_start(out=w1, in_=d["w1"])
            w2 = p3w.tile([128, 16 * 768], BF16)
            nc.sync.dma_start(out=w2, in_=d["w2"])

            def layer_norm_T(xin, gname, bname, yout):
                sq = p3f.tile([128, 6 * RC], BF16)
                nc.scalar.square(out=sq[:], in_=xin[:])
                s1 = p3st.tile([1, RC], F32)
                s2 = p3st.tile([1, RC], F32)
                for k in range(6):
                    nc.tensor.matmul(s1[:], ones_col[:],
                                     xin[:, k * RC:(k + 1) * RC],
                                     start=(k == 0), stop=(k == 5))
                for k in range(6):
                    nc.tensor.matmul(s2[:], ones_col[:],
                                     sq[:, k * RC:(k + 1) * RC],
                                     start=(k == 0), stop=(k == 5))
                mean = p3f.tile([1, RC], F32)
                nc.vector.tensor_scalar_mul(out=mean[:], in0=s1[:],
                                            scalar1=1.0 / D)
                var = p3f.tile([1, RC], F32)
                nc.vector.tensor_scalar_mul(out=var[:], in0=s2[:],
                                            scalar1=1.0 / D)
                msq = p3f.tile([1, RC], F32)
                nc.vector.tensor_tensor(out=msq[:], in0=mean[:], in1=mean[:],
                                        op=ALU.mult)
                nc.vector.tensor_tensor(out=var[:], in0=var[:], in1=msq[:],
                                        op=ALU.subtract)
                sd = p3f.tile([1, RC], F32)
                nc.scalar.activation(out=sd[:], in_=var[:], func=AF.Sqrt,
                                     bias=eps_t[:], scale=1.0)
                rstd = p3f.tile([1, RC], F32)
                nc.vector.reciprocal(out=rstd[:], in_=sd[:])
                nmr = p3f.tile([1, RC], F32)
                nc.vector.tensor_tensor(out=nmr[:], in0=mean[:], in1=rstd[:],
                                        op=ALU.mult)
                rstd_b = p3f.tile([1, RC], BF16)
                nc.vector.tensor_copy(out=rstd_b[:], in_=rstd[:])
                nmr_b = p3f.tile([1, RC], BF16)
                nc.vector.tensor_scalar_mul(out=nmr_b[:], in0=nmr[:], scalar1=-1.0)
                pa = p3ab.tile([128, RC], F32)
                nc.tensor.matmul(pa[:], ones_row[:], rstd_b[:],
                                 start=True, stop=True)
                pb = p3ab.tile([128, RC], F32)
                nc.tensor.matmul(pb[:], ones_row[:], nmr_b[:],
                                 start=True, stop=True)
                gv, bv = small[gname], small[bname]
                for k in range(6):
                    u = p3s.tile([128, RC], F32)
                    nc.vector.tensor_tensor(out=u[:],
                                            in0=xin[:, k * RC:(k + 1) * RC],
                                            in1=pa[:], op=ALU.mult)
                    if ln_triv:
                        nc.vector.tensor_tensor(out=yout(k), in0=u[:],
                                                in1=pb[:], op=ALU.add)
                    else:
                        nc.vector.tensor_tensor(out=u[:], in0=u[:], in1=pb[:],
                                                op=ALU.add)
                        nc.vector.tensor_scalar(out=yout(k), in0=u[:],
                                                scalar1=gv[:, k:k + 1],
                                                scalar2=bv[:, k:k + 1],
                                                op0=ALU.mult, op1=ALU.add)

            for c in range(NCHUNK):
                t2 = p3t.tile([128, 6 * RC], BF16)
                for m in range(6):
                    ps = p3ps.tile([128, RC], F32)
                    for kh in range(H):
                        nc.tensor.matmul(
                            ps[:],
                            wao[:, kh * 768 + m * 128:kh * 768 + m * 128 + 128],
                            oT[:, kh * R + c * RC:kh * R + (c + 1) * RC],
                            start=(kh == 0), stop=(kh == 7))
                    ta = p3s.tile([128, RC], BF16)
                    nc.scalar.activation(out=ta[:], in_=ps[:], func=AF.Identity,
                                         bias=small["bao"][:, m:m + 1], scale=1.0)
                    tg = small["tgt0"][:, m * 100:(m + 1) * 100]
                    nc.vector.tensor_tensor(out=t2[:, m * RC:(m + 1) * RC],
                                            in0=ta[:],
                                            in1=_ap(tg, [[0, 4], [1, 100]]),
                                            op=ALU.add)
                y2 = p3t.tile([128, 6 * RC], BF16)
                if ffn_triv:
                    # b1=b2=0 and trivial LN gains: relu is positive-
                    # homogeneous and LN3 is row-scale invariant, so LN2's
                    # rstd can be dropped entirely; center by mean only.
                    s1 = p3st.tile([1, RC], F32)
                    for k in range(6):
                        nc.tensor.matmul(s1[:], ones_col[:],
                                         t2[:, k * RC:(k + 1) * RC],
                                         start=(k == 0), stop=(k == 5))
                    nmean_b = p3f.tile([1, RC], BF16)
                    nc.vector.tensor_scalar_mul(out=nmean_b[:], in0=s1[:],
                                                scalar1=-1.0 / D)
                    pb = p3ab.tile([128, RC], F32)
                    nc.tensor.matmul(pb[:], ones_row[:], nmean_b[:],
                                     start=True, stop=True)
                    for k in range(6):
                        nc.vector.tensor_tensor(
                            out=y2[:, k * RC:(k + 1) * RC],
                            in0=t2[:, k * RC:(k + 1) * RC],
                            in1=pb[:], op=ALU.add)
                else:
                    layer_norm_T(t2, "ln2g", "ln2b",
                                 lambda k: y2[:, k * RC:(k + 1) * RC])
                ff1 = p3f.tile([128, 16 * RC], BF16)
                for mf in range(16):
                    ps = p3ps.tile([128, RC], F32)
                    for k in range(6):
                        nc.tensor.matmul(
                            ps[:],
                            w1[:, k * 2048 + mf * 128:k * 2048 + mf * 128 + 128],
                            y2[:, k * RC:(k + 1) * RC],
                            start=(k == 0), stop=(k == 5))
                    nc.scalar.activation(out=ff1[:, mf * RC:(mf + 1) * RC],
                                         in_=ps[:], func=AF.Relu,
                                         bias=small["b1"][:, mf:mf + 1],
                                         scale=1.0)
                t3 = p3t.tile([128, 6 * RC], BF16)
                for m in range(6):
                    ps = p3ps.tile([128, RC], F32)
                    for k in range(16):
                        nc.tensor.matmul(
                            ps[:],
                            w2[:, k * 768 + m * 128:k * 768 + m * 128 + 128],
                            ff1[:, k * RC:(k + 1) * RC],
                            start=(k == 0), stop=(k == 15))
                    tb = p3s.tile([128, RC], BF16)
                    nc.scalar.activation(out=tb[:], in_=ps[:], func=AF.Identity,
                                         bias=small["b2"][:, m:m + 1], scale=1.0)
                    nc.vector.tensor_tensor(out=t3[:, m * RC:(m + 1) * RC],
                                            in0=tb[:],
                                            in1=y2[:, m * RC:(m + 1) * RC],
                                            op=ALU.add)
                if ffn_triv:
                    # defer rstd3 to the GroupFC evacuation: center t3 only,
                    # stash rstd per row (scale commutes with h3 @ dup_g,
                    # dup_bias==0 guaranteed by the skip_dupb gate below)
                    sq = p3f.tile([128, 6 * RC], BF16)
                    nc.scalar.square(out=sq[:], in_=t3[:])
                    s1 = p3st.tile([1, RC], F32)
                    s2 = p3st.tile([1, RC], F32)
                    for k in range(6):
                        nc.tensor.matmul(s1[:], ones_col[:],
                                         t3[:, k * RC:(k + 1) * RC],
                                         start=(k == 0), stop=(k == 5))
                    for k in range(6):
                        nc.tensor.matmul(s2[:], ones_col[:],
                                         sq[:, k * RC:(k + 1) * RC],
                                         start=(k == 0), stop=(k == 5))
                    mean = p3f.tile([1, RC], F32)
                    nc.vector.tensor_scalar_mul(out=mean[:], in0=s1[:],
                                                scalar1=1.0 / D)
                    var = p3f.tile([1, RC], F32)
                    nc.vector.tensor_scalar_mul(out=var[:], in0=s2[:],
                                                scalar1=1.0 / D)
                    msq = p3f.tile([1, RC], F32)
                    nc.vector.tensor_tensor(out=msq[:], in0=mean[:],
                                            in1=mean[:], op=ALU.mult)
                    nc.vector.tensor_tensor(out=var[:], in0=var[:], in1=msq[:],
                                            op=ALU.subtract)
                    sd = p3f.tile([1, RC], F32)
                    nc.scalar.activation(out=sd[:], in_=var[:], func=AF.Sqrt,
                                         bias=eps_t[:], scale=1.0)
                    rstd = p3f.tile([1, RC], F32)
                    nc.vector.reciprocal(out=rstd[:], in_=sd[:])
                    nc.vector.tensor_copy(
                        out=rstd_all[:, c * RC:(c + 1) * RC], in_=rstd[:])
                    nmean_b = p3f.tile([1, RC], BF16)
                    nc.vector.tensor_scalar_mul(out=nmean_b[:], in0=s1[:],
                                                scalar1=-1.0 / D)
                    pb = p3ab.tile([128, RC], F32)
                    nc.tensor.matmul(pb[:], ones_row[:], nmean_b[:],
                                     start=True, stop=True)
                    for k in range(6):
                        nc.vector.tensor_tensor(
                            out=h3T[:, k * R + c * RC:k * R + (c + 1) * RC],
                            in0=t3[:, k * RC:(k + 1) * RC],
                            in1=pb[:], op=ALU.add)
                else:
                    layer_norm_T(t3, "ln3g", "ln3b",
                                 lambda k: h3T[:, k * R + c * RC:k * R + (c + 1) * RC])

        # ---- P4: GroupFC -> logitsT ----
        with pool("p4d", bufs=16) as p4d, pool("p4o") as p4o, \
             pool("p4rs_sb", bufs=2) as p4rs_sb, \
             pool("p4ps", bufs=2, space="PSUM") as p4ps, \
             pool("p4rs", bufs=2, space="PSUM") as p4rs:
            logitsT = p4o.tile([96, G * BL], F32)
            dupb = p4o.tile(list(d["dupb"].shape), BF16)
            nc.sync.dma_start(out=dupb, in_=d["dupb"])
            for g0 in range(0, G, 16):
                ng = min(16, G - g0)
                ps = p4ps.tile([96, 16 * BL], F32)
                for gi in range(ng):
                    g = g0 + gi
                    dup = p4d.tile([128, 6 * 96], BF16)
                    nc.sync.dma_start(out=dup, in_=d["dup"][g])
                    if not skip_dupb:
                        nc.tensor.matmul(ps[:, gi * BL:(gi + 1) * BL],
                                         dupb[:, g * 96:(g + 1) * 96],
                                         ones32[:], start=True, stop=False)
                    for k in range(6):
                        hsl = _ap(h3T[:, k * R + g:], [[100, BL]])
                        nc.tensor.matmul(ps[:, gi * BL:(gi + 1) * BL],
                                         dup[:, k * 96:(k + 1) * 96],
                                         hsl, start=(skip_dupb and k == 0),
                                         stop=(k == 5))
                if ffn_triv:
                    rs_ps = p4rs.tile([96, 16 * BL], F32)
                    rsl = rstd_all[:, g0:]
                    nc.tensor.matmul(
                        rs_ps[:, 0:ng * BL], ones_row[:, 0:96],
                        _ap(rsl, [[1, ng], [100, BL]]),
                        start=True, stop=True)
                    rs_sb = p4rs_sb.tile([96, 16 * BL], BF16)
                    nc.scalar.copy(out=rs_sb[:, 0:ng * BL],
                                   in_=rs_ps[:, 0:ng * BL])
                    nc.vector.tensor_tensor(
                        out=logitsT[:, g0 * BL:(g0 + ng) * BL],
                        in0=ps[:, 0:ng * BL], in1=rs_sb[:, 0:ng * BL],
                        op=ALU.mult)
                else:
                    nc.vector.tensor_copy(out=logitsT[:, g0 * BL:(g0 + ng) * BL],
                                          in_=ps[:, 0:ng * BL])
            nc.sync.dma_start(out=out_d, in_=logitsT[:])


_CACHE = {}


# ======================================================================
# Fast path: fp8 DoubleRow pipeline.
#
# Row order is g-major: r = g*32 + b (3200 rows/core, chunks of RC=512).
# All heavy matmuls run fp8e4 DoubleRow (0.5 cyc/row, 2x contraction per
# instr). Precision is preserved by splitting the FFN around the
# batch-independent query path: z0 = y0@w1, ff10 = relu(z0), and
# t30 = y0 + ff10@w2 are computed exactly on the host and re-injected
# with bf16 "indicator" matmuls; the device only quantizes the small
# batch-dependent delta (ow = o@wao, std ~0.1). LN2 centering is folded
# into w1 as a rank-1 host correction; the residual -mean(ow) term is
# dropped exactly (it vanishes in LN3). LN3 centering is folded into
# GroupFC via per-group colsum(dup) rank-1 matmuls; rstd is applied at
# the logits evacuation.
#
# Scales (powers of two): weights x128; qT x64; oT fp8 x8; ow bf16
# x4096; owq fp8 x16; psum1 = 2048*z; v = 32*ff1; dff fp8 = 32*dff;
# psum2/t3s = 4096*t3.
# ======================================================================

FP8 = mybir.dt.float8e4
F8NP = ml_dtypes.float8_e4m3fn
DR = mybir.MatmulPerfMode.DoubleRow
RC = 512
NCH = (R + RC - 1) // RC          # 7 (6x512 + 128)


def _f8(a, scale=1.0):
    a = np.asarray(a, np.float32) * scale
    return np.ascontiguousarray(np.clip(a, -240.0, 240.0).astype(F8NP))


def _bf16(a, scale=1.0):
    return np.ascontiguousarray((np.asarray(a, np.float32) * scale).astype(BF))


def build_fast(skip_dupb=True):
    nc = bacc.Bacc("TRN2", target_bir_lowering=False, debug=False,
                   num_devices=NCORES)
    d = {}

    def din(name, shape, dt):
        d[name] = nc.dram_tensor(name, list(shape), dt, kind="ExternalInput").ap()

    din("xT", (XCH, 128, 16 * 392), FP8)
    din("wemb", (128, 16 * 768), FP8)
    din("wk", (128, 6 * 768), FP8)
    din("wv", (128, 6 * 768), FP8)
    din("wao", (96, 8 * 768), FP8)
    din("qT", (96, 8 * 100), FP8)
    din("qbk", (100, 8), F32)
    din("be", (128, 6), F32)
    din("baos", (128, 6), F32)
    din("w1c", (128, 6 * 2048), FP8)
    din("w2", (128, 16 * 768), FP8)
    din("z0s", (100, 16 * 128), BF16)
    din("ff10s", (128, 16 * 100), BF16)
    din("t30s", (100, 6 * 128), BF16)
    din("dup", (G * 128, 576), BF16)
    out_d = nc.dram_tensor("logitsT", [96, R], F32, kind="ExternalOutput").ap()
    out_s1 = nc.dram_tensor("s1a", [1, R], F32, kind="ExternalOutput").ap()
    out_s2 = nc.dram_tensor("s2a", [1, R], F32, kind="ExternalOutput").ap()

    with tile.TileContext(nc) as tc:
        fast_kernel(tc, d, (out_d, out_s1, out_s2), skip_dupb)
    nc.compile()
    return nc


def fast_kernel(tc, d, outs, skip_dupb=True):
    out_d, out_s1, out_s2 = outs
    nc = tc.nc

    def pool(name, bufs=1, space="SBUF"):
        return tc.tile_pool(name=name, bufs=bufs, space=space)

    with pool("res") as res, pool("dupp", bufs=3) as dupp, \
         pool("oTp") as oTp:
        identb = res.tile([128, 128], BF16)
        make_identity(nc, identb[:])
        ones_col = res.tile([128, 1], BF16)
        nc.vector.memset(ones_col[:], 1.0)
        ones_row = res.tile([1, 128], BF16)
        nc.vector.memset(ones_row[:], 1.0)
        ones2f8 = res.tile([128, 32], FP8)
        nc.vector.memset(ones2f8[:], 1.0)

        small = {}
        names = ["qT", "qbk", "be", "baos", "z0s", "ff10s", "t30s"]
        for name in names:
            t = res.tile(list(d[name].shape), d[name].dtype, tag=name)
            nc.gpsimd.dma_start(out=t, in_=d[name])
            small[name] = t
        w1cq = res.tile([128, 6 * 2048], FP8)
        w2q = res.tile([128, 16 * 768], FP8)
        waoq = res.tile([96, 8 * 768], FP8)

        oTq = oTp.tile([96, 8 * R], FP8)

        CWs = [min(RC, R - c * RC) for c in range(NCH)]
        dup_halves = {}
        HALVES = []
        for blk in range(NCH):
            ngs = CWs[blk] // 32
            HALVES.append((blk, blk * 16, min(8, ngs)))
            if ngs > 8:
                HALVES.append((blk, blk * 16 + 8, ngs - 8))
        fetch_state = {"i": 0}

        def fetch_next_halves(n):
            for _ in range(n):
                i = fetch_state["i"]
                if i >= len(HALVES):
                    return
                fetch_state["i"] = i + 1
                blk, g0, ng = HALVES[i]
                bt = dupp.tile([128, 8 * 576], BF16, tag="dh")
                src = AP(tensor=d["dup"].tensor, offset=g0 * 128 * 576,
                         ap=[[576, 128], [128 * 576, ng], [1, 576]])
                nc.scalar.dma_start(out=_ap(bt[:], [[576, ng], [1, 576]]),
                                    in_=src)
                dup_halves[g0] = bt

        # ---------------- P0 + P1: mem, K^T, V ----------------
        with pool("kvp") as kvp:
            KTq = kvp.tile([96, 8 * 2048], FP8)
            Vq = kvp.tile([128, 16 * 768], BF16)
            with pool("memp") as memp, pool("p0w") as p0w, \
                 pool("p0x", bufs=2) as p0x, \
                 pool("p0ps", bufs=2, space="PSUM") as p0ps, \
                 pool("p1k", bufs=2, space="PSUM") as p1k, \
                 pool("p1v", bufs=2, space="PSUM") as p1v:
                memq = memp.tile([128, 6 * 2048], FP8)
                wembq = p0w.tile([128, 16 * 768], FP8)
                nc.scalar.dma_start(out=wembq, in_=d["wemb"])
                wkq = p0w.tile([128, 6 * 768], FP8)
                nc.scalar.dma_start(out=wkq, in_=d["wk"])
                wvq = p0w.tile([128, 6 * 768], FP8)
                nc.scalar.dma_start(out=wvq, in_=d["wv"])

                for c in range(XCH):
                    xq = p0x.tile([128, 16 * 392], FP8)
                    nc.sync.dma_start(out=xq, in_=d["xT"][c])
                    for m in range(6):
                        ps = p0ps.tile([128, 392], F32)
                        for kp in range(8):
                            nc.tensor.matmul(
                                ps[:],
                                _ap(wembq[:, 2 * kp * 768 + m * 128:],
                                    [[768, 2], [1, 128]]),
                                _ap(xq[:, 2 * kp * 392:], [[392, 2], [1, 392]]),
                                start=(kp == 0), stop=(kp == 7), perf_mode=DR)
                        nc.scalar.activation(
                            out=_ap(memq[:, m * 2048 + c * 512:],
                                    [[64, 8], [1, S]]),
                            in_=_ap(ps[:], [[S, 8], [1, S]]),
                            func=AF.Relu, bias=small["be"][:, m:m + 1],
                            scale=2.0 ** -7)

                # K^T (head-major, b-padded 64)
                for h in range(H):
                    for c in range(XCH):
                        ps = p1k.tile([96, 392], F32)
                        for kp in range(3):
                            nc.tensor.matmul(
                                ps[:],
                                _ap(wkq[:, 2 * kp * 768 + h * 96:],
                                    [[768, 2], [1, 96]]),
                                _ap(memq[:, 2 * kp * 2048 + c * 512:],
                                    [[2048, 2], [64, 8], [1, S]]),
                                start=(kp == 0), stop=(kp == 2), perf_mode=DR)
                        dst = _ap(KTq[:, h * 2048 + c * 512:], [[64, 8], [1, S]])
                        src = _ap(ps[:], [[S, 8], [1, S]])
                        if h % 2 == 0:
                            nc.scalar.activation(out=dst, in_=src, func=AF.Copy,
                                                 scale=2.0 ** -7)
                        else:
                            nc.vector.tensor_scalar_mul(out=dst, in0=src,
                                                        scalar1=2.0 ** -7)

                # V (rows b-pair padded on partitions)
                for t in range(16):
                    ps = p1v.tile([128, 768], F32)
                    for sub in range(2):
                        n0, n1 = sub * 512, min(768, (sub + 1) * 512)
                        for kp in range(3):
                            nc.tensor.matmul(
                                ps[:, n0:n1],
                                _ap(memq[:, 2 * kp * 2048 + t * 128:],
                                    [[2048, 2], [1, 128]]),
                                _ap(wvq[:, 2 * kp * 768 + n0:],
                                    [[768, 2], [1, n1 - n0]]),
                                start=(kp == 0), stop=(kp == 2), perf_mode=DR)
                    dst = Vq[:, t * 768:(t + 1) * 768]
                    if t % 2 == 0:
                        nc.vector.tensor_scalar_mul(out=dst, in0=ps[:],
                                                    scalar1=2.0 ** -7)
                    else:
                        nc.scalar.activation(out=dst, in_=ps[:], func=AF.Copy,
                                             scale=2.0 ** -7)

            nc.scalar.dma_start(out=waoq, in_=d["wao"])
            nc.scalar.dma_start(out=w1cq, in_=d["w1c"])
            nc.scalar.dma_start(out=w2q, in_=d["w2"])
            fetch_next_halves(3)

            # ---------------- P2: attention ----------------
            with pool("p2a", bufs=2) as p2a, pool("p2s", bufs=3) as p2s, \
                 pool("p2q", bufs=1) as p2q, \
                 pool("p2ps", bufs=2, space="PSUM") as psc, \
                 pool("p2pt", bufs=2, space="PSUM") as pst, \
                 pool("p2po", bufs=2, space="PSUM") as pso:
                for bg in range(4):
                    attnT = p2a.tile([128, 8 * 400], BF16)
                    hdat = []
                    for h in range(H):
                        ps = psc.tile([100, 392], F32)
                        nc.tensor.matmul(
                            ps[:], small["qT"][:, h * 100:(h + 1) * 100],
                            _ap(KTq[:, h * 2048 + bg * 512:], [[64, 8], [1, S]]),
                            start=True, stop=True)
                        att = p2s.tile([100, 8 * 64], BF16, tag="att")
                        nc.scalar.activation(
                            out=_ap(att[:], [[64, 8], [1, S]]), in_=ps[:],
                            func=AF.Exp, bias=small["qbk"][:, h:h + 1],
                            scale=2.0 ** -6)
                        sums = p2s.tile([100, 8], F32, tag="sums")
                        nc.vector.reduce_sum(out=sums[:],
                                             in_=_ap(att[:], [[64, 8], [1, S]]),
                                             axis=AX.X)
                        inv = p2s.tile([100, 8], BF16, tag="inv")
                        with nc.allow_low_precision(reason="softmax inv bf16"):
                            nc.vector.reciprocal(out=inv[:], in_=sums[:])
                        attnq = p2q.tile([100, 8 * 64], BF16, tag=f"aq{h}")
                        eng = nc.gpsimd if h % 2 == 0 else nc.vector
                        eng.tensor_tensor(
                            out=_ap(attnq[:], [[64, 8], [1, S]]),
                            in0=_ap(att[:], [[64, 8], [1, S]]),
                            in1=_ap(inv[:], [[1, 8], [0, S]]), op=ALU.mult)
                        hdat.append(attnq)
                    for h in range(H):
                        pt = pst.tile([128, 400], BF16)
                        for pr in range(4):
                            nc.tensor.transpose(
                                pt[:, pr * 100:(pr + 1) * 100],
                                hdat[h][:, pr * 128:(pr + 1) * 128],
                                identb[0:100, 0:100])
                        nc.scalar.activation(out=attnT[:, h * 400:(h + 1) * 400],
                                             in_=pt[:], func=AF.Copy)
                    for lb in range(8):
                        b = bg * 8 + lb
                        p0_ = (lb % 2) * 64
                        po = pso.tile([96, 1024], F32)
                        for h in range(H):
                            nc.tensor.matmul(
                                po[:, h * 128:h * 128 + 100],
                                Vq[p0_:p0_ + S,
                                   (b // 2) * 768 + h * 96:(b // 2) * 768 + h * 96 + 96],
                                attnT[p0_:p0_ + S,
                                      h * 400 + (lb // 2) * 100:
                                      h * 400 + (lb // 2) * 100 + 100],
                                start=True, stop=True)
                        dst = _ap(oTq[:, b:], [[R, 8], [32, 100]])
                        src_ = _ap(po[:], [[128, 8], [1, 100]])
                        if lb % 2 == 0:
                            nc.scalar.activation(out=dst, in_=src_, func=AF.Copy,
                                                 scale=8.0)
                        else:
                            nc.vector.tensor_scalar_mul(out=dst, in0=src_,
                                                        scalar1=8.0)

        # ---------------- P3 + P4: pipelined chunks ----------------
        # Per iteration (software pipeline, stage-shifted):
        #   C: FFN2 + LN3 stats for chunk it-2
        #   D: GroupFC + logits for chunk it-3
        #   A: attn_out for chunk it
        #   B: FFN1 -> dff (relu+subtract fused via max trick) for chunk it-1
        with pool("ows", bufs=3) as owsp, pool("owq", bufs=2) as owqp, \
             pool("dffp", bufs=2) as dffp, \
             pool("t3p", bufs=2) as t3p, pool("sqp", bufs=2) as sqp, \
             pool("smp", bufs=2) as smp, pool("logp", bufs=2) as logp, \
             pool("pmm", bufs=4, space="PSUM") as pmm, \
             pool("pstat", bufs=1, space="PSUM") as pstat, \
             pool("pp4", bufs=1, space="PSUM") as pp4:
            ows_t, owq_t, dff_t, t3_t = {}, {}, {}, {}

            for it in range(NCH + 3):
                # --- stage C: FFN2 + LN3 stats for chunk it-2 ---
                if 2 <= it < NCH + 2:
                    c = it - 2
                    cw, c0 = CWs[c], c * RC
                    dff = dff_t[c]
                    t3c = t3p.tile([128, 6 * RC], BF16, tag="t3")
                    for m in range(6):
                        ps = pmm.tile([128, RC], F32)
                        for kp in range(8):
                            nc.tensor.matmul(
                                ps[:, 0:cw],
                                _ap(w2q[:, 2 * kp * 768 + m * 128:],
                                    [[768, 2], [1, 128]]),
                                _ap(dff[:, 2 * kp * RC:], [[RC, 2], [1, cw]]),
                                start=(kp == 0), stop=False, perf_mode=DR)
                        nc.tensor.matmul(ps[:, 0:cw],
                                         small["t30s"][:, m * 128:(m + 1) * 128],
                                         _ap(identb[0:100, c * 16:],
                                             [[1, cw // 32], [0, 32]]),
                                         start=False, stop=True)
                        nc.vector.tensor_tensor(
                            out=t3c[:, m * RC:m * RC + cw], in0=ps[:, 0:cw],
                            in1=ows_t[c][:, m * RC:m * RC + cw], op=ALU.add)
                    t3_t[c] = t3c
                    # LN3 stats: s1 (bf16 ones matmuls), s2 (fp8 DR over sq)
                    s1 = pstat.tile([1, RC], F32)
                    for k in range(6):
                        nc.tensor.matmul(s1[:, 0:cw], ones_col[:],
                                         t3c[:, k * RC:k * RC + cw],
                                         start=(k == 0), stop=(k == 5))
                    sqc = sqp.tile([128, 6 * RC], FP8, tag="sq")
                    nc.scalar.activation(out=_ap(sqc[:], [[RC, 6], [1, cw]]),
                                         in_=_ap(t3c[:], [[RC, 6], [1, cw]]),
                                         func=AF.Square, scale=2.0 ** -12)
                    s2 = pstat.tile([16, RC], F32)
                    for kp in range(3):
                        nc.tensor.matmul(
                            s2[:, 0:cw], _ap(ones2f8[:], [[16, 2], [1, 16]]),
                            _ap(sqc[:, 2 * kp * RC:], [[RC, 2], [1, cw]]),
                            start=(kp == 0), stop=(kp == 2), perf_mode=DR)
                    s1sb = smp.tile([1, RC], F32, tag="s1")
                    nc.vector.tensor_copy(out=s1sb[:, 0:cw], in_=s1[:, 0:cw])
                    s2sb = smp.tile([1, RC], F32, tag="s2")
                    nc.vector.tensor_copy(out=s2sb[:, 0:cw], in_=s2[0:1, 0:cw])
                    nc.sync.dma_start(out=out_s1[:, c0:c0 + cw],
                                      in_=s1sb[:, 0:cw])
                    nc.sync.dma_start(out=out_s2[:, c0:c0 + cw],
                                      in_=s2sb[:, 0:cw])

                # --- stage D: GroupFC + logits for chunk it-3 ---
                if 3 <= it:
                    c = it - 3
                    cw, c0 = CWs[c], c * RC
                    t3c = t3_t[c]
                    ps4 = pp4.tile([96, RC], F32, tag="ps4", bufs=2)
                    for gi in range(cw // 32):
                        dblk = dup_halves[c * 16 + (gi // 8) * 8]
                        gl = gi % 8
                        for k in range(6):
                            nc.tensor.matmul(
                                ps4[:, gi * 32:(gi + 1) * 32],
                                dblk[:, gl * 576 + k * 96:gl * 576 + (k + 1) * 96],
                                t3c[:, k * RC + gi * 32:k * RC + gi * 32 + 32],
                                start=(k == 0), stop=(k == 5))
                    logc = logp.tile([96, RC], F32, tag="log")
                    nc.scalar.activation(out=logc[:, 0:cw], in_=ps4[:, 0:cw],
                                         func=AF.Copy)
                    nc.sync.dma_start(out=out_d[:, c0:c0 + cw],
                                      in_=logc[:, 0:cw])
                    dup_halves.pop(c * 16)
                    dup_halves.pop(c * 16 + 8, None)
                    fetch_next_halves(2)

                # --- stage A: attn_out for chunk it ---
                if it < NCH:
                    c = it
                    cw, c0 = CWs[c], c * RC
                    ows = owsp.tile([128, 6 * RC], BF16, tag="ows")
                    for m in range(6):
                        ps = pmm.tile([128, RC], F32)
                        for hp in range(4):
                            nc.tensor.matmul(
                                ps[:, 0:cw],
                                _ap(waoq[:, 2 * hp * 768 + m * 128:],
                                    [[768, 2], [1, 128]]),
                                _ap(oTq[:, 2 * hp * R + c0:], [[R, 2], [1, cw]]),
                                start=(hp == 0), stop=(hp == 3), perf_mode=DR)
                        nc.scalar.activation(out=ows[:, m * RC:m * RC + cw],
                                             in_=ps[:, 0:cw], func=AF.Identity,
                                             bias=small["baos"][:, m:m + 1],
                                             scale=4.0)
                    owq = owqp.tile([128, 6 * RC], FP8, tag="owq")
                    nc.gpsimd.tensor_scalar_mul(
                        out=_ap(owq[:], [[RC, 6], [1, cw]]),
                        in0=_ap(ows[:], [[RC, 6], [1, cw]]), scalar1=2.0 ** -8)
                    ows_t[c], owq_t[c] = ows, owq

                # --- stage B: FFN1 -> dff for chunk it-1 ---
                # psum1 = 64*(dz + min(z0,0)); dff = max(psum1, -64*ff10)
                # equals 64*(relu(z0+dz) - relu(z0)) exactly.
                if 1 <= it < NCH + 1:
                    c = it - 1
                    cw, c0 = CWs[c], c * RC
                    owq = owq_t[c]
                    dff = dffp.tile([128, 16 * RC], FP8, tag="dff")
                    for mf in range(16):
                        ps = pmm.tile([128, RC], F32)
                        for kp in range(3):
                            nc.tensor.matmul(
                                ps[:, 0:cw],
                                _ap(w1cq[:, 2 * kp * 2048 + mf * 128:],
                                    [[2048, 2], [1, 128]]),
                                _ap(owq[:, 2 * kp * RC:], [[RC, 2], [1, cw]]),
                                start=(kp == 0), stop=False, perf_mode=DR)
                        nc.tensor.matmul(
                            ps[:, 0:cw],
                            small["z0s"][:, mf * 128:(mf + 1) * 128],
                            _ap(identb[0:100, c * 16:],
                                [[1, cw // 32], [0, 32]]),
                            start=False, stop=True)
                        nc.vector.tensor_tensor(
                            out=dff[:, mf * RC:mf * RC + cw], in0=ps[:, 0:cw],
                            in1=_ap(small["ff10s"][:, mf * 100 + c * 16:],
                                    [[1, cw // 32], [0, 32]]),
                            op=ALU.max)
                    dff_t[c] = dff


def _prep_fast(inputs, skip_dupb):
    f32 = lambda k: np.asarray(inputs[k], np.float32)
    x = f32("x")
    w_qkv, b_qkv = f32("w_qkv"), f32("b_qkv")
    w_attn_out, b_attn_out = f32("w_attn_out"), f32("b_attn_out")
    w1, w2, dup = f32("w1"), f32("w2"), f32("dup_pool")

    # batch-independent query path (host, exact)
    t = 2.0 * f32("query_embed")
    mu = t.mean(-1, keepdims=True)
    va = ((t - mu) ** 2).mean(-1, keepdims=True)
    tgt0 = (t - mu) / np.sqrt(va + EPS) * f32("ln1_g") + f32("ln1_b")
    q = (tgt0 @ w_qkv[:, :D] + b_qkv[:D]) / np.sqrt(float(HD))
    bk = b_qkv[D:2 * D]
    qbk = np.stack([q[:, h * HD:(h + 1) * HD] @ bk[h * HD:(h + 1) * HD]
                    for h in range(H)], axis=1)
    bv = b_qkv[2 * D:]
    bao_eff = b_attn_out + bv @ w_attn_out

    y0 = tgt0 - tgt0.mean(-1, keepdims=True)
    w1c = w1 - np.ones((D, 1), np.float32) @ (w1.sum(0, keepdims=True) / D)
    z0 = y0 @ w1
    ff10 = np.maximum(z0, 0.0)
    t30 = y0 + ff10 @ w2
    # host-side LN3 epilogue constants
    rr = np.arange(R)
    csumT = np.ascontiguousarray(dup.sum(1).T[:, rr // BL])     # [96, R]

    col6 = lambda a: np.ascontiguousarray(a.reshape(6, 128).T.astype(np.float32))
    feed = {
        "wemb": _f8(f32("w_embed").reshape(16, 128, 768).transpose(1, 0, 2)
                    .reshape(128, -1), 128.0),
        "wk": _f8(w_qkv[:, D:2 * D].reshape(6, 128, 768).transpose(1, 0, 2)
                  .reshape(128, -1), 128.0),
        "wv": _f8(w_qkv[:, 2 * D:].reshape(6, 128, 768).transpose(1, 0, 2)
                  .reshape(128, -1), 128.0),
        "wao": _f8(w_attn_out.reshape(8, 96, 768).transpose(1, 0, 2)
                   .reshape(96, -1), 128.0),
        "qT": _f8(q.T.reshape(8, 96, 100).transpose(1, 0, 2).reshape(96, -1),
                  64.0),
        "qbk": np.ascontiguousarray(qbk.astype(np.float32)),
        "be": col6(f32("b_embed")),
        "baos": col6(bao_eff * 4096.0),
        "w1c": _f8(w1c.reshape(6, 128, 2048).transpose(1, 0, 2)
                   .reshape(128, -1), 4.0),
        "w2": _f8(w2.reshape(16, 128, 768).transpose(1, 0, 2)
                  .reshape(128, -1), 64.0),
        "z0s": _bf16(np.minimum(z0, 0.0).reshape(100, 16 * 128), 64.0),
        "ff10s": _bf16(-ff10.reshape(100, 16, 128).transpose(2, 1, 0)
                       .reshape(128, -1), 64.0),
        "t30s": _bf16(t30.reshape(100, 6 * 128), 4096.0),
        "dup": _bf16(dup.reshape(G, 6, 128, 96).transpose(0, 2, 1, 3)
                     .reshape(G * 128, 6 * 96)),
    }

    xr = x.reshape(NCORES, XCH, 8, S, 16, 128)
    in_maps = []
    for core in range(NCORES):
        xT = xr[core].transpose(0, 4, 3, 1, 2).reshape(XCH, 128, 16 * 392)
        in_maps.append({**feed, "xT": _f8(xT)})
    host = {"csumT": csumT, "dupb": f32("dup_bias")}
    return in_maps, host


def kernel(**inputs):
    f32 = lambda k: np.asarray(inputs[k], np.float32)
    fast_ok = bool(
        np.all(f32("ln2_g") == 1.0) and np.all(f32("ln2_b") == 0.0)
        and np.all(f32("ln3_g") == 1.0) and np.all(f32("ln3_b") == 0.0)
        and np.all(f32("b1") == 0.0) and np.all(f32("b2") == 0.0))
    if fast_ok:
        skip_dupb = bool(np.all(f32("dup_bias") == 0.0))
        key = ("fast", skip_dupb)
        if key not in _CACHE:
            _CACHE[key] = build_fast(skip_dupb)
        nc = _CACHE[key]
        _CACHE["nc"] = nc
        in_maps, host = _prep_fast(inputs, skip_dupb)
        _CACHE["in_maps"] = in_maps
        res = run_bass_kernel_spmd(nc, in_maps, list(range(NCORES)))
        csumT, dupb = host["csumT"], host["dupb"]
        outs = []
        for core in range(NCORES):
            r_ = res.results[core]
            lt = np.asarray(r_["logitsT"], np.float32)
            s1a = np.asarray(r_["s1a"], np.float32).reshape(-1)
            s2a = np.asarray(r_["s2a"], np.float32).reshape(-1)
            m = s1a / (4096.0 * 768.0)
            var = s2a / 768.0 - m * m
            rstd = 1.0 / np.sqrt(var + EPS)
            ltn = (lt * (1.0 / 4096.0) - csumT * m[None, :]) * rstd[None, :]
            outs.append(ltn.reshape(96, G, BL).transpose(2, 1, 0)
                        .reshape(BL, G * DF))
        out = np.concatenate(outs, axis=0)
        if not skip_dupb:
            out = out + dupb[None, :]
        return out.astype(np.float32)

    x = f32("x")
    w_qkv, b_qkv = f32("w_qkv"), f32("b_qkv")
    w_attn_out, b_attn_out = f32("w_attn_out"), f32("b_attn_out")

    # host constant folding for the batch-independent query path
    t = 2.0 * f32("query_embed")
    mu = t.mean(-1, keepdims=True)
    va = ((t - mu) ** 2).mean(-1, keepdims=True)
    tgt0 = (t - mu) / np.sqrt(va + EPS) * f32("ln1_g") + f32("ln1_b")
    q = (tgt0 @ w_qkv[:, :D] + b_qkv[:D]) / np.sqrt(float(HD))
    bk = b_qkv[D:2 * D]
    qbk = np.stack([q[:, h * HD:(h + 1) * HD] @ bk[h * HD:(h + 1) * HD]
                    for h in range(H)], axis=1)
    bv = b_qkv[2 * D:]
    bao_eff = b_attn_out + bv @ w_attn_out   # softmax rows sum to 1

    col6 = lambda a: np.ascontiguousarray(a.reshape(6, 128).T)
    feed = {
        "wemb": _bf(f32("w_embed").reshape(16, 128, 768).transpose(1, 0, 2)
                    .reshape(128, -1)),
        "be": col6(f32("b_embed")),
        "wk": _bf(w_qkv[:, D:2 * D].reshape(6, 128, 768).transpose(1, 0, 2)
                  .reshape(128, -1)),
        "wv": _bf(w_qkv[:, 2 * D:].reshape(6, 128, 768).transpose(1, 0, 2)
                  .reshape(128, -1)),
        "wao": _bf(w_attn_out.reshape(8, 96, 768).transpose(1, 0, 2)
                   .reshape(96, -1)),
        "bao": col6(bao_eff),
        "w1": _bf(f32("w1").reshape(6, 128, 2048).transpose(1, 0, 2)
                  .reshape(128, -1)),
        "b1": np.ascontiguousarray(f32("b1").reshape(16, 128).T),
        "w2": _bf(f32("w2").reshape(16, 128, 768).transpose(1, 0, 2)
                  .reshape(128, -1)),
        "b2": col6(f32("b2")),
        "qT": _bf(q.T.reshape(8, 96, 100).transpose(1, 0, 2).reshape(96, -1)),
        "qbk": np.ascontiguousarray(qbk.astype(np.float32)),
        "tgt0": _bf(tgt0.T.reshape(6, 128, 100).transpose(1, 0, 2)
                    .reshape(128, -1)),
        "ln2g": col6(f32("ln2_g")), "ln2b": col6(f32("ln2_b")),
        "ln3g": col6(f32("ln3_g")), "ln3b": col6(f32("ln3_b")),
        "dup": _bf(f32("dup_pool").reshape(G, 6, 128, 96).transpose(0, 2, 1, 3)
                   .reshape(G, 128, 6 * 96)),
        "dupb": _bf(f32("dup_bias").reshape(1, -1)),
    }

    skip_dupb = bool(np.all(f32("dup_bias") == 0.0))
    ln_triv = bool(np.all(f32("ln2_g") == 1.0) and np.all(f32("ln2_b") == 0.0)
                   and np.all(f32("ln3_g") == 1.0) and np.all(f32("ln3_b") == 0.0))
    ffn_triv = bool(ln_triv and np.all(f32("b1") == 0.0)
                    and np.all(f32("b2") == 0.0))
    key = ("nc", skip_dupb, ln_triv, ffn_triv)
    if key not in _CACHE:
        _CACHE[key] = build_program(skip_dupb, ln_triv, ffn_triv)
    nc = _CACHE[key]
    _CACHE["nc"] = nc

    # xr[core] axes: [c, col, k, p]; device wants [c, p, k, col]
    xr = x.reshape(NCORES, XCH, XCOLS, 16, 128)
    in_maps = []
    for core in range(NCORES):
        xT = xr[core].transpose(0, 3, 2, 1).reshape(XCH, 128, 16 * XCOLS)
        in_maps.append({**feed, "xT": _bf(xT)})

    _CACHE["in_maps"] = in_maps
    res = run_bass_kernel_spmd(nc, in_maps, list(range(NCORES)))
    outs = []
    for core in range(NCORES):
        lt = np.asarray(res.results[core]["logitsT"], np.float32)
        outs.append(lt.reshape(96, G, BL).transpose(2, 1, 0).reshape(BL, G * DF))
    return np.concatenate(outs, axis=0).astype(np.float32)

